# revision 4
# baseline (speedup 1.0000x reference)
"""Trainium2 Bass kernel for a dense transformer block.

Shapes (hardcoded from the problem spec):
  x [B=8, P=576, D=1024], H=16 heads, HD=64, HID=4096.

Sharding: data-parallel over batch. Core i processes batch element i
(576 tokens); weights are replicated to every core; no collectives.

Per-core dataflow (token-major activations, bf16 matmuls, f32 residuals):
  LN1 -> QKV matmul -> QK-LayerNorm -> per-head attention
  (S^T = k @ q^T via PE, exp on ACT without max-subtraction -- QK-LN
  bounds |logits| <= 8 -- then O = exp(S^T)^T @ [v|ones] which yields the
  softmax numerator and denominator in a single matmul) -> o-LN -> proj
  (+residual; ls1 folded into w_proj on host) -> LN2 -> MLP with
  tanh-approx GELU (ls2 folded into w2) -> residual.

Transposes between token-major and feature-major layouts are done by
bouncing bf16 tensors through DRAM and re-loading with the DMA XBAR
transpose (per-128-column chunk tensors so reads only depend on the
writes of that chunk).
"""

import sys

if "/opt/trn_rl_repo" not in sys.path:
    sys.path.insert(0, "/opt/trn_rl_repo")

import math
from contextlib import ExitStack

import ml_dtypes
import numpy as np

import concourse.bass as bass
import concourse.bacc as bacc
import concourse.mybir as mybir
import concourse.tile as tile

F32 = mybir.dt.float32
BF16 = mybir.dt.bfloat16
AX = mybir.AxisListType.X
OP = mybir.AluOpType
ACTF = mybir.ActivationFunctionType

D = 1024
H = 16
HD = 64
HID = 4096
EPS = 1e-6
N_CORES = 8


def _ttiles(T):
    return [(i * 128, min(128, T - i * 128)) for i in range(math.ceil(T / 128))]


def _nsplit(n, chunk=512):
    out = []
    o = 0
    while o < n:
        out.append((o, min(chunk, n - o)))
        o += chunk
    return out


def build_block(T=576, flags=None, gelu=ACTF.Gelu_apprx_tanh):
    """Builds the Bass program for one core (T tokens). Returns nc.

    flags: dict of booleans for which optional vector inputs are
    non-trivial and must be applied on-device:
      ln1, ln2, oln (gamma/beta), qk (q/k gamma/beta), bproj, b1, b2
    """
    flags = flags or {}
    nc = bacc.Bacc("TRN2", target_bir_lowering=False, debug=False)

    TT = _ttiles(T)
    NT = len(TT)
    KD = D // 128  # 8 k-chunks over D
    KH = HID // 128  # 32 k-chunks over HID

    # ---------------- DRAM I/O ----------------
    x_d = nc.dram_tensor("x", [T, D], F32, kind="ExternalInput")
    wqkv_d = nc.dram_tensor("wqkv", [D, 3 * D], BF16, kind="ExternalInput")
    wproj_d = nc.dram_tensor("wproj", [D, D], BF16, kind="ExternalInput")
    w1_d = nc.dram_tensor("w1", [D, HID], BF16, kind="ExternalInput")
    w2_d = nc.dram_tensor("w2", [HID, D], BF16, kind="ExternalInput")
    out_d = nc.dram_tensor("out", [T, D], F32, kind="ExternalOutput")

    opt_d = {}
    for name, shape, want in [
        ("ln1_g", [D], flags.get("ln1")),
        ("ln1_b", [D], flags.get("ln1")),
        ("ln2_g", [D], flags.get("ln2")),
        ("ln2_b", [D], flags.get("ln2")),
        ("o_g", [D], flags.get("oln")),
        ("o_b", [D], flags.get("oln")),
        ("qg", [HD], flags.get("qk")),
        ("qb", [HD], flags.get("qk")),
        ("kg", [HD], flags.get("qk")),
        ("kb", [HD], flags.get("qk")),
        ("bproj", [D], flags.get("bproj")),
        ("b1", [HID], flags.get("b1")),
        ("b2", [D], flags.get("b2")),
    ]:
        if want:
            opt_d[name] = nc.dram_tensor(name, shape, F32, kind="ExternalInput")

    with tile.TileContext(nc) as tc, ExitStack() as ctx:
        # ---------------- pools ----------------
        persist = ctx.enter_context(tc.tile_pool(name="persist", bufs=1))
        dram = ctx.enter_context(tc.tile_pool(name="dram", bufs=1, space="DRAM"))
        stats = ctx.enter_context(tc.tile_pool(name="stats", bufs=4))
        ev = ctx.enter_context(tc.tile_pool(name="ev", bufs=3))

        # constants
        eps_t = persist.tile([128, 1], F32, tag="eps")
        nc.vector.memset(eps_t, EPS)

        # replicated optional vectors
        rep = {}
        for name, width in [
            ("ln1_g", D), ("ln1_b", D), ("ln2_g", D), ("ln2_b", D),
            ("o_g", D), ("o_b", D), ("qg", HD), ("qb", HD),
            ("kg", HD), ("kb", HD), ("bproj", D), ("b1", HID), ("b2", D),
        ]:
            if name in opt_d:
                t = persist.tile([128, width], F32, tag=f"rep_{name}")
                nc.sync.dma_start(out=t, in_=opt_d[name][:].to_broadcast([128, width]))
                rep[name] = t

        # DRAM bounce tensors, chunked by 128 feature columns
        xln_dc = [dram.tile([T, 128], BF16, tag=f"xln{c}", name=f"xln{c}") for c in range(KD)]
        qln_dc = [dram.tile([T, 128], BF16, tag=f"qln{c}", name=f"qln{c}") for c in range(KD)]
        kln_dc = [dram.tile([T, 128], BF16, tag=f"kln{c}", name=f"kln{c}") for c in range(KD)]
        oln_dc = [dram.tile([T, 128], BF16, tag=f"oln{c}", name=f"oln{c}") for c in range(KD)]
        ln2_dc = [dram.tile([T, 128], BF16, tag=f"ln2{c}", name=f"ln2{c}") for c in range(KD)]
        h_dc = [dram.tile([T, 512], BF16, tag=f"h{c}", name=f"h{c}") for c in range(HID // 512)]

        # persistent SBUF
        xres = persist.tile([128, NT, D], F32, tag="xres")
        r1 = persist.tile([128, NT, D], F32, tag="r1")
        qkv = persist.tile([128, NT, 3 * D], BF16, tag="qkv")
        v_aug = persist.tile([128, NT, H, HD + 1], BF16, tag="v_aug")
        attn = persist.tile([128, NT, D], BF16, tag="attn")

        # ---------- helper: token-major layernorm over D ----------
        def ln_tile(src_ap, tp, dst_bf16, gname):
            """dst = LN(src) (* g + b if flagged). src [tp, D] f32/bf16."""
            st = stats.tile([128, 2, nc.vector.BN_STATS_DIM], F32, tag="ln_st")
            for s in range(2):
                nc.vector.bn_stats(
                    out=st[:tp, s], in_=src_ap[:, s * 512:(s + 1) * 512]
                )
            mv = stats.tile([128, nc.vector.BN_AGGR_DIM], F32, tag="ln_mv")
            nc.vector.bn_aggr(out=mv[:tp], in_=st[:tp])
            sd = stats.tile([128, 1], F32, tag="ln_sd")
            nc.scalar.activation(
                out=sd[:tp], in_=mv[:tp, 1:2], func=ACTF.Sqrt, bias=eps_t[:tp]
            )
            rstd = stats.tile([128, 1], F32, tag="ln_rstd")
            nc.vector.reciprocal(out=rstd[:tp], in_=sd[:tp])
            nc.vector.tensor_scalar(
                out=dst_bf16,
                in0=src_ap,
                scalar1=mv[:tp, 0:1],
                scalar2=rstd[:tp],
                op0=OP.subtract,
                op1=OP.mult,
            )
            if gname in rep:
                nc.vector.tensor_mul(dst_bf16, dst_bf16, rep[gname][:tp])
                nc.vector.tensor_add(
                    dst_bf16, dst_bf16, rep[gname.replace("_g", "_b")][:tp]
                )

        # ================= Stage 1: load x, LN1 =================
        for ti, (t0, tp) in enumerate(TT):
            nc.sync.dma_start(out=xres[:tp, ti], in_=x_d[t0:t0 + tp])
        for ti, (t0, tp) in enumerate(TT):
            xln_t = ev.tile([128, D], BF16, tag="xln_t")
            ln_tile(xres[:tp, ti], tp, xln_t[:tp], "ln1_g")
            for c in range(KD):
                nc.sync.dma_start(
                    out=xln_dc[c][t0:t0 + tp], in_=xln_t[:tp, c * 128:(c + 1) * 128]
                )

        # ================= Stage 2: QKV matmul =================
        with tc.tile_pool(name="qkv_w", bufs=3) as wpool, \
             tc.tile_pool(name="qkv_ps", bufs=2, space="PSUM") as pspool, \
             tc.tile_pool(name="lhsT", bufs=1) as lhsT_pool:
            xT = lhsT_pool.tile([128, KD, T], BF16, tag="xT")
            for c in range(KD):
                nc.sync.dma_start(out=xT[:, c], in_=xln_dc[c][:], transpose=True)
            wq_r = wqkv_d[:].rearrange("(ko p) n -> p ko n", p=128)
            for n0, nw in _nsplit(3 * D):
                wt = wpool.tile([128, KD, 512], BF16, tag="wt")
                nc.sync.dma_start(out=wt[:, :, :nw], in_=wq_r[:, :, n0:n0 + nw])
                for ti, (t0, tp) in enumerate(TT):
                    ps = pspool.tile([128, 512], F32, tag="ps")
                    for k in range(KD):
                        nc.tensor.matmul(
                            ps[:tp, :nw],
                            xT[:, k, t0:t0 + tp],
                            wt[:, k, :nw],
                            start=(k == 0),
                            stop=(k == KD - 1),
                        )
                    nc.any.tensor_copy(
                        out=qkv[:tp, ti, n0:n0 + nw], in_=ps[:tp, :nw]
                    )

        # ================= Stage 3: QK-LN, build v_aug =================
        inv_hd = 1.0 / HD
        scale = HD ** (-0.5)
        for ti, (t0, tp) in enumerate(TT):
            # v | ones
            nc.vector.memset(v_aug[:tp, ti, :, HD:], 1.0)
            if tp < 128:
                nc.vector.memset(v_aug[tp:, ti, :, HD:], 0.0)
            nc.gpsimd.tensor_copy(
                out=v_aug[:tp, ti, :, :HD],
                in_=qkv[:tp, ti, 2 * D:3 * D].rearrange("p (h d) -> p h d", h=H),
            )
            for which, base, dst_dc, gkey in (
                ("q", 0, qln_dc, "qg"),
                ("k", D, kln_dc, "kg"),
            ):
                src = qkv[:tp, ti, base:base + D].rearrange(
                    "p (h d) -> p h d", h=H
                )
                sq = stats.tile([128, H, HD], F32, tag="qk_sq")
                nc.vector.tensor_mul(sq[:tp], src, src)
                s1 = stats.tile([128, H], F32, tag="qk_s1")
                nc.vector.reduce_sum(out=s1[:tp], in_=src, axis=AX)
                s2 = stats.tile([128, H], F32, tag="qk_s2")
                nc.vector.reduce_sum(out=s2[:tp], in_=sq[:tp], axis=AX)
                mean = stats.tile([128, H], F32, tag="qk_mean")
                nc.vector.tensor_scalar_mul(mean[:tp], s1[:tp], inv_hd)
                msq = stats.tile([128, H], F32, tag="qk_msq")
                nc.vector.tensor_mul(msq[:tp], mean[:tp], mean[:tp])
                var = stats.tile([128, H], F32, tag="qk_var")
                nc.vector.tensor_scalar(
                    out=var[:tp], in0=s2[:tp], scalar1=inv_hd, scalar2=None,
                    op0=OP.mult,
                )
                nc.vector.tensor_sub(var[:tp], var[:tp], msq[:tp])
                sd = stats.tile([128, H], F32, tag="qk_sd")
                nc.scalar.activation(
                    out=sd[:tp], in_=var[:tp], func=ACTF.Sqrt, bias=eps_t[:tp]
                )
                rstd = stats.tile([128, H], F32, tag="qk_rstd")
                nc.vector.reciprocal(out=rstd[:tp], in_=sd[:tp])
                if which == "q" and "qg" not in rep:
                    nc.vector.tensor_scalar_mul(rstd[:tp], rstd[:tp], scale)
                cen = stats.tile([128, H, HD], F32, tag="qk_cen")
                nc.vector.tensor_tensor(
                    cen[:tp], src,
                    mean[:tp, :, None].to_broadcast([tp, H, HD]), OP.subtract,
                )
                lnt = ev.tile([128, H, HD], BF16, tag="qk_out")
                nc.vector.tensor_tensor(
                    lnt[:tp], cen[:tp],
                    rstd[:tp, :, None].to_broadcast([tp, H, HD]), OP.mult,
                )
                if gkey in rep:
                    g = rep[gkey]
                    b = rep["qb" if which == "q" else "kb"]
                    nc.vector.tensor_tensor(
                        lnt[:tp], lnt[:tp],
                        g[:tp, None, :].to_broadcast([tp, H, HD]), OP.mult,
                    )
                    nc.vector.tensor_tensor(
                        lnt[:tp], lnt[:tp],
                        b[:tp, None, :].to_broadcast([tp, H, HD]), OP.add,
                    )
                    if which == "q":
                        nc.vector.tensor_scalar_mul(lnt[:tp], lnt[:tp], scale)
                flat = lnt[:tp].rearrange("p h d -> p (h d)")
                for c in range(KD):
                    nc.sync.dma_start(
                        out=dst_dc[c][t0:t0 + tp],
                        in_=flat[:, c * 128:(c + 1) * 128],
                    )

        # ================= Stage 4: attention =================
        with tc.tile_pool(name="att_qkT", bufs=1) as qkT_pool, \
             tc.tile_pool(name="att_exp", bufs=2) as exp_pool, \
             tc.tile_pool(name="att_ps", bufs=2, space="PSUM") as qk_ps_pool, \
             tc.tile_pool(name="av_ps", bufs=3, space="PSUM") as av_ps_pool:
            qT = qkT_pool.tile([128, KD, T], BF16, tag="qT")
            kT = qkT_pool.tile([128, KD, T], BF16, tag="kT")
            for c in range(KD):
                nc.sync.dma_start(out=qT[:, c], in_=qln_dc[c][:], transpose=True)
                nc.sync.dma_start(out=kT[:, c], in_=kln_dc[c][:], transpose=True)
            for h in range(H):
                c, off = h // 2, (h % 2) * 64
                q_h = qT[off:off + 64, c]
                k_h = kT[off:off + 64, c]
                exp_tiles = []
                for tk, (tk0, tkw) in enumerate(TT):
                    ps = qk_ps_pool.tile([128, T], F32, tag="qk_ps")
                    for n0, nw in _nsplit(T):
                        nc.tensor.matmul(
                            ps[:tkw, n0:n0 + nw],
                            k_h[:, tk0:tk0 + tkw],
                            q_h[:, n0:n0 + nw],
                            start=True,
                            stop=True,
                        )
                    et = exp_pool.tile([128, T], BF16, tag=f"exp{tk}")
                    nc.scalar.activation(out=et[:tkw], in_=ps[:tkw], func=ACTF.Exp)
                    exp_tiles.append(et)
                for mi, (m0, mp) in enumerate(TT):
                    pso = av_ps_pool.tile([128, HD + 1], F32, tag="av_ps")
                    for tk, (tk0, tkw) in enumerate(TT):
                        nc.tensor.matmul(
                            pso[:mp],
                            exp_tiles[tk][:tkw, m0:m0 + mp],
                            v_aug[:tkw, tk, h],
                            start=(tk == 0),
                            stop=(tk == NT - 1),
                        )
                    rc = stats.tile([128, 1], F32, tag="att_rc")
                    nc.vector.reciprocal(out=rc[:mp], in_=pso[:mp, HD:])
                    nc.vector.tensor_scalar_mul(
                        attn[:mp, mi, h * HD:(h + 1) * HD], pso[:mp, :HD], rc[:mp]
                    )

        # ================= Stage 5: o-LN =================
        for ti, (t0, tp) in enumerate(TT):
            ot = ev.tile([128, D], BF16, tag="oln_t")
            ln_tile(attn[:tp, ti], tp, ot[:tp], "o_g")
            for c in range(KD):
                nc.sync.dma_start(
                    out=oln_dc[c][t0:t0 + tp], in_=ot[:tp, c * 128:(c + 1) * 128]
                )

        # ================= Stage 6: proj + residual =================
        with tc.tile_pool(name="proj_w", bufs=2) as wpool, \
             tc.tile_pool(name="proj_ps", bufs=2, space="PSUM") as pspool, \
             tc.tile_pool(name="proj_lhsT", bufs=1) as lhsT_pool:
            oT = lhsT_pool.tile([128, KD, T], BF16, tag="oT")
            for c in range(KD):
                nc.sync.dma_start(out=oT[:, c], in_=oln_dc[c][:], transpose=True)
            wp_r = wproj_d[:].rearrange("(ko p) n -> p ko n", p=128)
            for n0, nw in _nsplit(D):
                wt = wpool.tile([128, KD, 512], BF16, tag="wt")
                nc.sync.dma_start(out=wt[:, :, :nw], in_=wp_r[:, :, n0:n0 + nw])
                for ti, (t0, tp) in enumerate(TT):
                    ps = pspool.tile([128, 512], F32, tag="ps")
                    for k in range(KD):
                        nc.tensor.matmul(
                            ps[:tp, :nw],
                            oT[:, k, t0:t0 + tp],
                            wt[:, k, :nw],
                            start=(k == 0),
                            stop=(k == KD - 1),
                        )
                    dst = r1[:tp, ti, n0:n0 + nw]
                    nc.vector.tensor_add(dst, ps[:tp, :nw], xres[:tp, ti, n0:n0 + nw])
                    if "bproj" in rep:
                        nc.vector.tensor_add(
                            dst, dst, rep["bproj"][:tp, n0:n0 + nw]
                        )

        # ================= Stage 7: LN2 =================
        for ti, (t0, tp) in enumerate(TT):
            lt = ev.tile([128, D], BF16, tag="ln2_t")
            ln_tile(r1[:tp, ti], tp, lt[:tp], "ln2_g")
            for c in range(KD):
                nc.sync.dma_start(
                    out=ln2_dc[c][t0:t0 + tp], in_=lt[:tp, c * 128:(c + 1) * 128]
                )

        # ================= Stage 8: MLP1 + gelu =================
        with tc.tile_pool(name="m1_w", bufs=3) as wpool, \
             tc.tile_pool(name="m1_ps", bufs=2, space="PSUM") as pspool, \
             tc.tile_pool(name="m1_lhsT", bufs=1) as lhsT_pool, \
             tc.tile_pool(name="m1_out", bufs=3) as hpool:
            l2T = lhsT_pool.tile([128, KD, T], BF16, tag="l2T")
            for c in range(KD):
                nc.sync.dma_start(out=l2T[:, c], in_=ln2_dc[c][:], transpose=True)
            w1_r = w1_d[:].rearrange("(ko p) n -> p ko n", p=128)
            for ni, (n0, nw) in enumerate(_nsplit(HID)):
                wt = wpool.tile([128, KD, 512], BF16, tag="wt")
                nc.sync.dma_start(out=wt[:], in_=w1_r[:, :, n0:n0 + nw])
                for ti, (t0, tp) in enumerate(TT):
                    ps = pspool.tile([128, 512], F32, tag="ps")
                    for k in range(KD):
                        nc.tensor.matmul(
                            ps[:tp],
                            l2T[:, k, t0:t0 + tp],
                            wt[:, k],
                            start=(k == 0),
                            stop=(k == KD - 1),
                        )
                    ht = hpool.tile([128, 512], BF16, tag="ht")
                    if "b1" in rep:
                        tmp = hpool.tile([128, 512], F32, tag="ht_tmp")
                        nc.vector.tensor_add(
                            tmp[:tp], ps[:tp], rep["b1"][:tp, n0:n0 + nw]
                        )
                        nc.scalar.activation(
                            out=ht[:tp], in_=tmp[:tp], func=gelu
                        )
                    else:
                        nc.scalar.activation(
                            out=ht[:tp], in_=ps[:tp], func=gelu
                        )
                    nc.sync.dma_start(out=h_dc[ni][t0:t0 + tp], in_=ht[:tp])

        # ================= Stage 9: MLP2 + residual =================
        with tc.tile_pool(name="m2_w", bufs=2) as wpool, \
             tc.tile_pool(name="m2_ps", bufs=1, space="PSUM") as pspool, \
             tc.tile_pool(name="m2_lhsT", bufs=1) as lhsT_pool, \
             tc.tile_pool(name="m2_out", bufs=2) as opool:
            hT = lhsT_pool.tile([128, KH, T], BF16, tag="hT")
            for ni in range(HID // 512):
                for j in range(4):
                    nc.sync.dma_start(
                        out=hT[:, ni * 4 + j],
                        in_=h_dc[ni][:, j * 128:(j + 1) * 128],
                        transpose=True,
                    )
            w2_r = w2_d[:].rearrange("(ko p) n -> p ko n", p=128)
            for n0, nw in _nsplit(D):
                ps_tiles = [
                    pspool.tile([128, 512], F32, tag=f"ps{ti}", name=f"m2ps{ti}") for ti in range(NT)
                ]
                for kg in range(KH // 8):
                    wt = wpool.tile([128, 8, 512], BF16, tag="wt")
                    nc.sync.dma_start(
                        out=wt[:], in_=w2_r[:, kg * 8:(kg + 1) * 8, n0:n0 + nw]
                    )
                    for ti, (t0, tp) in enumerate(TT):
                        for k in range(8):
                            nc.tensor.matmul(
                                ps_tiles[ti][:tp],
                                hT[:, kg * 8 + k, t0:t0 + tp],
                                wt[:, k],
                                start=(kg == 0 and k == 0),
                                stop=(kg == KH // 8 - 1 and k == 7),
                            )
                for ti, (t0, tp) in enumerate(TT):
                    ot = opool.tile([128, 512], F32, tag="ot")
                    nc.vector.tensor_add(
                        ot[:tp], ps_tiles[ti][:tp], r1[:tp, ti, n0:n0 + nw]
                    )
                    if "b2" in rep:
                        nc.vector.tensor_add(
                            ot[:tp], ot[:tp], rep["b2"][:tp, n0:n0 + nw]
                        )
                    nc.sync.dma_start(
                        out=out_d[t0:t0 + tp, n0:n0 + nw], in_=ot[:tp]
                    )

    nc.compile()
    return nc


def _nontrivial(a, val):
    return not np.allclose(a, val, rtol=0.0, atol=0.0)


def prepare(inputs):
    """Host-side preprocessing: dtype casts and scale folding.

    Returns (flags, common_map) where common_map holds every device input
    except per-core "x".
    """
    f32 = np.float32
    bf16 = ml_dtypes.bfloat16
    w_qkv = np.asarray(inputs["w_qkv"], f32)
    w_proj = np.asarray(inputs["w_proj"], f32)
    w1 = np.asarray(inputs["w1"], f32)
    w2 = np.asarray(inputs["w2"], f32)
    ls1 = np.asarray(inputs["ls1"], f32)
    ls2 = np.asarray(inputs["ls2"], f32)

    flags = {
        "ln1": _nontrivial(inputs["ln1_g"], 1) or _nontrivial(inputs["ln1_b"], 0),
        "ln2": _nontrivial(inputs["ln2_g"], 1) or _nontrivial(inputs["ln2_b"], 0),
        "oln": _nontrivial(inputs["o_g"], 1) or _nontrivial(inputs["o_b"], 0),
        "qk": _nontrivial(inputs["q_g"], 1) or _nontrivial(inputs["q_b"], 0)
        or _nontrivial(inputs["k_g"], 1) or _nontrivial(inputs["k_b"], 0),
        "bproj": _nontrivial(inputs["b_proj"] * ls1, 0),
        "b1": _nontrivial(inputs["b1"], 0),
        "b2": _nontrivial(inputs["b2"] * ls2, 0),
    }

    scale = HD ** (-0.5)
    cm = {
        "wqkv": w_qkv.astype(bf16),
        "wproj": (w_proj * ls1[None, :]).astype(bf16),
        "w1": w1.astype(bf16),
        "w2": (w2 * ls2[None, :]).astype(bf16),
    }
    if flags["ln1"]:
        cm["ln1_g"] = np.asarray(inputs["ln1_g"], f32)
        cm["ln1_b"] = np.asarray(inputs["ln1_b"], f32)
    if flags["ln2"]:
        cm["ln2_g"] = np.asarray(inputs["ln2_g"], f32)
        cm["ln2_b"] = np.asarray(inputs["ln2_b"], f32)
    if flags["oln"]:
        cm["o_g"] = np.asarray(inputs["o_g"], f32)
        cm["o_b"] = np.asarray(inputs["o_b"], f32)
    if flags["qk"]:
        cm["qg"] = np.asarray(inputs["q_g"], f32)
        cm["qb"] = np.asarray(inputs["q_b"], f32)
        cm["kg"] = np.asarray(inputs["k_g"], f32)
        cm["kb"] = np.asarray(inputs["k_b"], f32)
    if flags["bproj"]:
        cm["bproj"] = (np.asarray(inputs["b_proj"], f32) * ls1).astype(f32)
    if flags["b1"]:
        cm["b1"] = np.asarray(inputs["b1"], f32)
    if flags["b2"]:
        cm["b2"] = (np.asarray(inputs["b2"], f32) * ls2).astype(f32)
    return flags, cm


_CACHE = {}


def get_compiled(flags):
    key = tuple(sorted((k, bool(v)) for k, v in flags.items()))
    if key not in _CACHE:
        _CACHE[key] = build_block(T=576, flags=flags)
    return _CACHE[key]


def kernel(**inputs):
    from concourse import bass_utils

    x = np.asarray(inputs["x"], np.float32)
    B = x.shape[0]
    assert B == N_CORES
    flags, cm = prepare(inputs)
    nc = get_compiled(flags)
    in_maps = [dict(cm, x=np.ascontiguousarray(x[i])) for i in range(B)]
    res = bass_utils.run_bass_kernel_spmd(nc, in_maps, core_ids=list(range(B)))
    out = np.stack([res.results[i]["out"] for i in range(B)], axis=0)
    return out.astype(np.float32)


if __name__ == "__main__":
    import reference

    inputs = {k: np.asarray(v) for k, v in reference.setup_inputs().items()}
    expected = np.asarray(reference.reference(**reference.setup_inputs()))
    actual = kernel(**inputs)
    err = np.linalg.norm(actual - expected) / np.linalg.norm(expected)
    print("Relative error:", err)


# revision 37
# speedup vs baseline: 1.5887x; 1.5887x over previous
"""Trainium2 Bass kernel for a dense transformer block.

Shapes (hardcoded from the problem spec):
  x [B=8, P=576, D=1024], H=16 heads, HD=64, HID=4096.

Sharding: data-parallel over batch. Core i processes batch element i
(576 tokens); weights are replicated to every core; no collectives.

Per-core dataflow (token-major activations, bf16 matmuls, f32 residuals):
  LN1 -> QKV matmul -> QK-LayerNorm -> per-head attention
  (S^T = k @ q^T via PE, exp on ACT without max-subtraction -- QK-LN
  bounds |logits| <= 8 -- then O = exp(S^T)^T @ [v|ones] which yields the
  softmax numerator and denominator in a single matmul) -> o-LN -> proj
  (+residual; ls1 folded into w_proj on host) -> LN2 -> MLP1 emitted
  feature-major (so GELU output IS the transposed input of MLP2, no
  DRAM bounce; b1 becomes a per-partition ACT bias) -> MLP2 -> residual.

Token-major <-> feature-major transposes bounce bf16 tensors through
DRAM and re-load with single XBAR transpose-DMA instructions (3D out).
"""

import sys

if "/opt/trn_rl_repo" not in sys.path:
    sys.path.insert(0, "/opt/trn_rl_repo")

import math
from contextlib import ExitStack

import ml_dtypes
import numpy as np

import concourse.bass as bass
import concourse.bacc as bacc
import concourse.mybir as mybir
import concourse.tile as tile

F32 = mybir.dt.float32
BF16 = mybir.dt.bfloat16
FP8 = mybir.dt.float8e4
W_SCALE = 16.0  # host multiplies fp8 weights by this; descaled at eviction
AX = mybir.AxisListType.X
OP = mybir.AluOpType
ACTF = mybir.ActivationFunctionType

D = 1024
H = 16
HD = 64
HID = 4096
EPS = 1e-6
N_CORES = 8


def _ttiles(T):
    return [(i * 128, min(128, T - i * 128)) for i in range(math.ceil(T / 128))]


def _nsplit(n, chunk=512):
    out = []
    o = 0
    while o < n:
        out.append((o, min(chunk, n - o)))
        o += chunk
    return out


class _StopBuild(Exception):
    pass


def build_block(T=576, flags=None, gelu=ACTF.Gelu_apprx_tanh, fp8=True, dr=True):
    """Builds the Bass program for one core (T tokens). Returns nc.

    flags: dict of booleans for which optional vector inputs are
    non-trivial and must be applied on-device:
      ln1, ln2, oln (gamma/beta), qk (q/k gamma/beta), bproj, b1, b2
    """
    flags = flags or {}
    nc = bacc.Bacc("TRN2", target_bir_lowering=False, debug=False)
    fp8 = bool(fp8)

    TT = _ttiles(T)
    NT = len(TT)
    KD = D // 128  # 8 k-chunks over D
    KH = HID // 128  # 32 k-chunks over HID

    # ---------------- DRAM I/O ----------------
    x_d = nc.dram_tensor("x", [T, D], F32, kind="ExternalInput")
    wdt = FP8 if fp8 else BF16
    wqkv_d = nc.dram_tensor("wqkv", [D, 3 * D], wdt, kind="ExternalInput")
    wproj_d = nc.dram_tensor("wproj", [D, D], wdt, kind="ExternalInput")
    w1_d = nc.dram_tensor("w1", [D, HID], wdt, kind="ExternalInput")
    w2_d = nc.dram_tensor("w2", [HID, D], wdt, kind="ExternalInput")
    out_d = nc.dram_tensor("out", [T, D], F32, kind="ExternalOutput")

    opt_d = {}
    for name, shape, want in [
        ("ln1_g", [D], flags.get("ln1")),
        ("ln1_b", [D], flags.get("ln1")),
        ("ln2_g", [D], flags.get("ln2")),
        ("ln2_b", [D], flags.get("ln2")),
        ("o_g", [D], flags.get("oln")),
        ("o_b", [D], flags.get("oln")),
        ("qg", [HD], flags.get("qk")),
        ("qb", [HD], flags.get("qk")),
        ("kg", [HD], flags.get("qk")),
        ("kb", [HD], flags.get("qk")),
        ("bproj", [D], flags.get("bproj")),
        ("b1", [HID], flags.get("b1")),
        ("b2", [D], flags.get("b2")),
        ("ls1s", [1], fp8 and flags.get("ls1u", True)),
        ("ls2s", [1], fp8 and flags.get("ls2u", True)),
        ("ls1v", [D], fp8 and not flags.get("ls1u", True)),
        ("ls2v", [D], fp8 and not flags.get("ls2u", True)),
    ]:
        if want:
            opt_d[name] = nc.dram_tensor(name, shape, F32, kind="ExternalInput")

    try:
        _build_body(nc, T, flags, gelu, x_d, wqkv_d, wproj_d, w1_d, w2_d,
                    out_d, opt_d, fp8, dr)
    except _StopBuild:
        pass
    nc.compile()
    return nc


def _build_body(nc, T, flags, gelu, x_d, wqkv_d, wproj_d, w1_d, w2_d,
                out_d, opt_d, fp8, dr):
    WDT = FP8 if fp8 else BF16
    DRM = mybir.MatmulPerfMode.DoubleRow if (fp8 and dr) else None
    KSTEP = 2 if (fp8 and dr) else 1
    descale = (1.0 / W_SCALE) if fp8 else 1.0
    TT = _ttiles(T)
    NT = len(TT)
    KD = D // 128
    KH = HID // 128
    with tile.TileContext(nc) as tc, ExitStack() as ctx:
        # ---------------- pools ----------------
        persist = ctx.enter_context(tc.tile_pool(name="persist", bufs=1))
        dram = ctx.enter_context(tc.tile_pool(name="dram", bufs=1, space="DRAM"))
        stats = ctx.enter_context(tc.tile_pool(name="stats", bufs=4))
        ev = ctx.enter_context(tc.tile_pool(name="ev", bufs=2))

        # constants
        eps_t = persist.tile([128, 1], F32, tag="eps")
        nc.vector.memset(eps_t, EPS)

        # replicated optional vectors (broadcast over partitions)
        rep = {}
        for name, width in [
            ("ln1_g", D), ("ln1_b", D), ("ln2_g", D), ("ln2_b", D),
            ("o_g", D), ("o_b", D), ("qg", HD), ("qb", HD),
            ("kg", HD), ("kb", HD), ("bproj", D), ("b2", D),
        ]:
            if name in opt_d:
                t = persist.tile([128, width], F32, tag=f"rep_{name}")
                nc.sync.dma_start(out=t, in_=opt_d[name][:].to_broadcast([128, width]))
                rep[name] = t

        # per-partition ls1/ls2 descale scalars (fp8 path)
        ls_sb = {}
        for nm in ("ls1s", "ls2s"):
            if nm in opt_d:
                t = persist.tile([128, 1], F32, tag=f"ls_{nm}")
                nc.sync.dma_start(out=t, in_=opt_d[nm][:].to_broadcast([128, 1]))
                ls_sb[nm] = t
        for nm in ("ls1v", "ls2v"):
            if nm in opt_d:
                t = persist.tile([128, D], F32, tag=f"ls_{nm}")
                nc.sync.dma_start(out=t, in_=opt_d[nm][:].to_broadcast([128, D]))
                ls_sb[nm] = t

        # DRAM bounce tensors (token-major; re-read via XBAR transpose DMA)
        xln_d = dram.tile([T, D], BF16, tag="xln_d")
        qln_d = dram.tile([T, D], BF16, tag="qln_d")
        kln_d = dram.tile([T, D], BF16, tag="kln_d")
        oln_d = dram.tile([T, D], BF16, tag="oln_d")
        ln2_d = dram.tile([T, D], BF16, tag="ln2_d")

        r1 = persist.tile([128, NT, D], F32, tag="r1")

        # ---------- helper: token-major layernorm over D ----------
        def ln_tile(src_ap, tp, dst_bf16, gname):
            """dst = LN(src) (* g + b if flagged). src [tp, D] f32/bf16."""
            st = stats.tile([128, 2, nc.vector.BN_STATS_DIM], F32, tag="ln_st", bufs=2)
            for s in range(2):
                nc.vector.bn_stats(
                    out=st[:tp, s], in_=src_ap[:, s * 512:(s + 1) * 512]
                )
            mv = stats.tile([128, nc.vector.BN_AGGR_DIM], F32, tag="ln_mv")
            nc.vector.bn_aggr(out=mv[:tp], in_=st[:tp])
            sd = stats.tile([128, 1], F32, tag="ln_sd")
            nc.scalar.activation(
                out=sd[:tp], in_=mv[:tp, 1:2], func=ACTF.Sqrt, bias=eps_t[:tp]
            )
            rstd = stats.tile([128, 1], F32, tag="ln_rstd")
            nc.vector.reciprocal(out=rstd[:tp], in_=sd[:tp])
            negmr = stats.tile([128, 1], F32, tag="ln_negmr")
            nc.vector.tensor_tensor(
                negmr[:tp], mv[:tp, 0:1], rstd[:tp], OP.mult
            )
            nc.vector.tensor_scalar_mul(negmr[:tp], negmr[:tp], -1.0)
            nc.scalar.activation(
                out=dst_bf16,
                in_=src_ap,
                func=ACTF.Identity,
                bias=negmr[:tp],
                scale=rstd[:tp],
            )
            if gname in rep:
                nc.vector.tensor_mul(dst_bf16, dst_bf16, rep[gname][:tp])
                nc.vector.tensor_add(
                    dst_bf16, dst_bf16, rep[gname.replace("_g", "_b")][:tp]
                )

        stop_after = flags.get("stop_after", 99)
        with tc.tile_pool(name="blk1", bufs=1) as blk1:
            xres = blk1.tile([128, NT, D], F32, tag="xres")
            qkv_cm = tc.tile_pool(name="qkv_sb", bufs=1)
            qkv_pool = qkv_cm.__enter__()
            qkv = qkv_pool.tile([128, NT, 3 * D], BF16, tag="qkv")
            v_aug = blk1.tile([128, NT, H, HD + 1], BF16, tag="v_aug")
            attn = blk1.tile([128, NT, D], BF16, tag="attn")
            rk_sb = blk1.tile([128, NT, H], F32, tag="rk_sb")

            # ================= Stage 1: load x, LN1 =================
            nfull = (T // 128) * 128
            nc.sync.dma_start(
                out=xres[:, :T // 128],
                in_=x_d[:nfull].rearrange("(t p) d -> p t d", p=128),
            )
            if nfull < T:
                nc.sync.dma_start(out=xres[:T - nfull, NT - 1], in_=x_d[nfull:T])
            for ti, (t0, tp) in enumerate(TT):
                xln_t = ev.tile([128, D], BF16, tag="xln_t")
                ln_tile(xres[:tp, ti], tp, xln_t[:tp], "ln1_g")
                nc.scalar.dma_start(out=xln_d[t0:t0 + tp], in_=xln_t[:tp])

            if stop_after <= 1:
                raise _StopBuild
            # ================= Stage 2: QKV matmul =================
            with tc.tile_pool(name="qkv_w", bufs=6) as wpool, \
                 tc.tile_pool(name="qkv_ps", bufs=4, space="PSUM") as pspool, \
                 tc.tile_pool(name="lhsT", bufs=1) as lhsT_pool:
                xT = lhsT_pool.tile([128, KD, T], BF16, tag="xT")
                if fp8:
                    xT8 = lhsT_pool.tile([128, KD, T], WDT, tag="xT8")
                else:
                    xT8 = xT
                for t0, tp in TT:
                    nc.sync.dma_start(
                        out=xT[:, :, t0:t0 + tp], in_=xln_d[t0:t0 + tp],
                        transpose=True,
                    )
                    if fp8:
                        nc.gpsimd.tensor_copy(
                            out=xT8[:, :, t0:t0 + tp], in_=xT[:, :, t0:t0 + tp]
                        )
                wq_r = wqkv_d[:].rearrange("(ko p) n -> p ko n", p=128)
                for n0, nw in _nsplit(3 * D):
                    wt = wpool.tile([128, KD, 512], WDT, tag="wt")
                    nc.sync.dma_start(out=wt[:, :, :nw], in_=wq_r[:, :, n0:n0 + nw])
                    for ti, (t0, tp) in enumerate(TT):
                        ps = pspool.tile([128, 512], F32, tag="ps")
                        for k in range(0, KD, KSTEP):
                            nc.tensor.matmul(
                                ps[:tp, :nw],
                                xT8[:, k:k + KSTEP, t0:t0 + tp],
                                wt[:, k:k + KSTEP, :nw],
                                start=(k == 0),
                                stop=(k == KD - KSTEP),
                                perf_mode=DRM,
                            )
                        if fp8:
                            nc.any.tensor_scalar_mul(
                                qkv[:tp, ti, n0:n0 + nw], ps[:tp, :nw], descale
                            )
                        else:
                            nc.any.tensor_copy(
                                out=qkv[:tp, ti, n0:n0 + nw], in_=ps[:tp, :nw]
                            )

            if stop_after <= 2:
                raise _StopBuild
            # ================= Stage 3: QK-LN, build v_aug =================
            inv_hd = 1.0 / HD
            scale = HD ** (-0.5)
            for ti, (t0, tp) in enumerate(TT):
                # v | ones
                nc.vector.memset(v_aug[:tp, ti, :, HD:], 1.0)
                if tp < 128:
                    nc.vector.memset(v_aug[tp:, ti, :, HD:], 0.0)
                nc.gpsimd.tensor_copy(
                    out=v_aug[:tp, ti, :, :HD],
                    in_=qkv[:tp, ti, 2 * D:3 * D].rearrange("p (h d) -> p h d", h=H),
                )
                qk_fast = "qg" not in rep
                for which, base, dst_d, gkey in (
                    ("q", 0, qln_d, "qg"),
                    ("k", D, kln_d, "kg"),
                ):
                    src = qkv[:tp, ti, base:base + D].rearrange(
                        "p (h d) -> p h d", h=H
                    )
                    sq = stats.tile([128, H, HD], F32, tag="qk_sq", bufs=1)
                    nc.gpsimd.tensor_mul(sq[:tp], src, src)
                    s1 = stats.tile([128, H], F32, tag="qk_s1")
                    nc.vector.reduce_sum(out=s1[:tp], in_=src, axis=AX)
                    s2 = stats.tile([128, H], F32, tag="qk_s2")
                    nc.vector.reduce_sum(out=s2[:tp], in_=sq[:tp], axis=AX)
                    mean = stats.tile([128, H], F32, tag="qk_mean")
                    nc.vector.tensor_scalar_mul(mean[:tp], s1[:tp], inv_hd)
                    msq = stats.tile([128, H], F32, tag="qk_msq")
                    nc.vector.tensor_mul(msq[:tp], mean[:tp], mean[:tp])
                    var = stats.tile([128, H], F32, tag="qk_var")
                    nc.vector.tensor_scalar(
                        out=var[:tp], in0=s2[:tp], scalar1=inv_hd, scalar2=None,
                        op0=OP.mult,
                    )
                    nc.vector.tensor_sub(var[:tp], var[:tp], msq[:tp])
                    sd = stats.tile([128, H], F32, tag="qk_sd")
                    nc.scalar.activation(
                        out=sd[:tp], in_=var[:tp], func=ACTF.Sqrt, bias=eps_t[:tp]
                    )
                    lnt = ev.tile([128, H, HD], BF16, tag="qk_out")
                    if qk_fast:
                        # k is centered only (rk folded into exp scale);
                        # q is scaled by rstd*hd^-0.5 only (its mean term
                        # vanishes against centered k).
                        if which == "k":
                            nc.vector.reciprocal(
                                out=rk_sb[:tp, ti], in_=sd[:tp]
                            )
                            nc.gpsimd.tensor_tensor(
                                lnt[:tp], src,
                                mean[:tp, :, None].to_broadcast([tp, H, HD]),
                                OP.subtract,
                            )
                        else:
                            rq = stats.tile([128, H], F32, tag="qk_rq")
                            nc.vector.reciprocal(out=rq[:tp], in_=sd[:tp])
                            nc.vector.tensor_scalar_mul(rq[:tp], rq[:tp], scale)
                            nc.vector.tensor_tensor(
                                lnt[:tp], src,
                                rq[:tp, :, None].to_broadcast([tp, H, HD]),
                                OP.mult,
                            )
                    else:
                        rstd = stats.tile([128, H], F32, tag="qk_rstd")
                        nc.vector.reciprocal(out=rstd[:tp], in_=sd[:tp])
                        nc.gpsimd.tensor_tensor(
                            lnt[:tp], src,
                            mean[:tp, :, None].to_broadcast([tp, H, HD]),
                            OP.subtract,
                        )
                        nc.vector.tensor_tensor(
                            lnt[:tp], lnt[:tp],
                            rstd[:tp, :, None].to_broadcast([tp, H, HD]), OP.mult,
                        )
                        g = rep[gkey]
                        b = rep["qb" if which == "q" else "kb"]
                        nc.vector.tensor_tensor(
                            lnt[:tp], lnt[:tp],
                            g[:tp, None, :].to_broadcast([tp, H, HD]), OP.mult,
                        )
                        nc.vector.tensor_tensor(
                            lnt[:tp], lnt[:tp],
                            b[:tp, None, :].to_broadcast([tp, H, HD]), OP.add,
                        )
                        if which == "q":
                            nc.vector.tensor_scalar_mul(lnt[:tp], lnt[:tp], scale)
                    flat = lnt[:tp].rearrange("p h d -> p (h d)")
                    nc.scalar.dma_start(out=dst_d[t0:t0 + tp], in_=flat)

            # qkv dead; free its SBUF and start MLP weight streams into it
            qkv_cm.__exit__(None, None, None)
            mlp_w = ctx.enter_context(
                tc.tile_pool(name="mlp_w", bufs=1, side="right")
            )
            w1_r = w1_d[:].rearrange("(ko p) n -> p ko n", p=128)
            w1_tiles = []
            for ni, (n0, nw) in enumerate(_nsplit(HID)):
                wt = mlp_w.tile([128, KD, 512], WDT, tag=f"w1_{ni}",
                                name=f"w1t{ni}")
                nc.sync.dma_start(out=wt[:], in_=w1_r[:, :, n0:n0 + nw])
                w1_tiles.append(wt)
            wt2 = mlp_w.tile([128, KH, D], WDT, tag="wt2")
            nc.sync.dma_start(
                out=wt2[:], in_=w2_d[:].rearrange("(ko p) n -> p ko n", p=128)
            )

            # ================= Stage 4: attention =================
            with tc.tile_pool(name="att_qkT", bufs=1) as qkT_pool, \
                 tc.tile_pool(name="att_exp", bufs=2) as exp_pool, \
                 tc.tile_pool(name="att_ps", bufs=3, space="PSUM") as qk_ps_pool, \
                 tc.tile_pool(name="av_ps", bufs=2, space="PSUM") as av_ps_pool:
                qT = qkT_pool.tile([128, KD, T], BF16, tag="qT")
                kT = qkT_pool.tile([128, KD, T], BF16, tag="kT")
                nc.scalar.dma_start(out=qT[:], in_=qln_d[:], transpose=True)
                nc.scalar.dma_start(out=kT[:], in_=kln_d[:], transpose=True)
                def qk_exp(h):
                    c, off = h // 2, (h % 2) * 64
                    q_h = qT[off:off + 64, c]
                    k_h = kT[off:off + 64, c]
                    exp_tiles = []
                    for tk, (tk0, tkw) in enumerate(TT):
                        ps = qk_ps_pool.tile(
                            [128, T], F32, tag="qk_ps", name=f"qkps{h}_{tk}"
                        )
                        for n0, nw in _nsplit(T):
                            nc.tensor.matmul(
                                ps[:tkw, n0:n0 + nw],
                                k_h[:, tk0:tk0 + tkw],
                                q_h[:, n0:n0 + nw],
                                start=True,
                                stop=True,
                            )
                        et = exp_pool.tile(
                            [128, T], BF16, tag=f"exp{tk}", name=f"exp{h}_{tk}"
                        )
                        if "qg" not in rep:
                            nc.scalar.activation(
                                out=et[:tkw], in_=ps[:tkw], func=ACTF.Exp,
                                scale=rk_sb[:tkw, tk, h:h + 1],
                            )
                        else:
                            nc.scalar.activation(
                                out=et[:tkw], in_=ps[:tkw], func=ACTF.Exp
                            )
                        exp_tiles.append(et)
                    return exp_tiles

                def av(h, exp_tiles):
                    for mi, (m0, mp) in enumerate(TT):
                        pso = av_ps_pool.tile(
                            [128, HD + 1], F32, tag="av_ps", name=f"avps{h}_{mi}"
                        )
                        for tk, (tk0, tkw) in enumerate(TT):
                            nc.tensor.matmul(
                                pso[:mp],
                                exp_tiles[tk][:tkw, m0:m0 + mp],
                                v_aug[:tkw, tk, h],
                                start=(tk == 0),
                                stop=(tk == NT - 1),
                            )
                        rc = stats.tile([128, 1], F32, tag="att_rc")
                        nc.vector.reciprocal(out=rc[:mp], in_=pso[:mp, HD:])
                        nc.vector.tensor_scalar_mul(
                            attn[:mp, mi, h * HD:(h + 1) * HD],
                            pso[:mp, :HD], rc[:mp],
                        )

                prev = None
                for h in range(H):
                    cur = qk_exp(h)
                    if prev is not None:
                        av(h - 1, prev)
                    prev = cur
                av(H - 1, prev)

            # ================= Stage 5: o-LN =================
            for ti, (t0, tp) in enumerate(TT):
                ot = ev.tile([128, D], BF16, tag="oln_t")
                ln_tile(attn[:tp, ti], tp, ot[:tp], "o_g")
                nc.scalar.dma_start(out=oln_d[t0:t0 + tp], in_=ot[:tp])

            if stop_after <= 5:
                raise _StopBuild
            # ================= Stage 6: proj + residual =================
            with tc.tile_pool(name="proj_w", bufs=2) as wpool, \
                 tc.tile_pool(name="proj_ps", bufs=4, space="PSUM") as pspool, \
                 tc.tile_pool(name="proj_lhsT", bufs=1) as lhsT_pool:
                oT = lhsT_pool.tile([128, KD, T], BF16, tag="oT")
                if fp8:
                    oT8 = lhsT_pool.tile([128, KD, T], WDT, tag="oT8")
                else:
                    oT8 = oT
                for t0, tp in TT:
                    nc.scalar.dma_start(
                        out=oT[:, :, t0:t0 + tp], in_=oln_d[t0:t0 + tp],
                        transpose=True,
                    )
                    if fp8:
                        nc.gpsimd.tensor_copy(
                            out=oT8[:, :, t0:t0 + tp], in_=oT[:, :, t0:t0 + tp]
                        )
                wp_r = wproj_d[:].rearrange("(ko p) n -> p ko n", p=128)
                for n0, nw in _nsplit(D):
                    wt = wpool.tile([128, KD, 512], WDT, tag="wt")
                    nc.sync.dma_start(out=wt[:, :, :nw], in_=wp_r[:, :, n0:n0 + nw])
                    for ti, (t0, tp) in enumerate(TT):
                        ps = pspool.tile([128, 512], F32, tag="ps")
                        for k in range(0, KD, KSTEP):
                            nc.tensor.matmul(
                                ps[:tp, :nw],
                                oT8[:, k:k + KSTEP, t0:t0 + tp],
                                wt[:, k:k + KSTEP, :nw],
                                start=(k == 0),
                                stop=(k == KD - KSTEP),
                                perf_mode=DRM,
                            )
                        dst = r1[:tp, ti, n0:n0 + nw]
                        if fp8:
                            if "ls1s" in opt_d:
                                nc.scalar.activation(
                                    out=dst, in_=ps[:tp, :nw],
                                    func=ACTF.Identity,
                                    scale=ls_sb["ls1s"][:tp],
                                )
                            else:
                                nc.vector.tensor_mul(
                                    dst, ps[:tp, :nw],
                                    ls_sb["ls1v"][:tp, n0:n0 + nw],
                                )
                            nc.vector.tensor_add(
                                dst, dst, xres[:tp, ti, n0:n0 + nw]
                            )
                        else:
                            nc.vector.tensor_add(
                                dst, ps[:tp, :nw], xres[:tp, ti, n0:n0 + nw]
                            )
                        if "bproj" in rep:
                            nc.vector.tensor_add(
                                dst, dst, rep["bproj"][:tp, n0:n0 + nw]
                            )

        if stop_after <= 6:
            raise _StopBuild
        # ================= Stage 7: LN2 =================
        for ti, (t0, tp) in enumerate(TT):
            lt = ev.tile([128, D], BF16, tag="ln2_t")
            ln_tile(r1[:tp, ti], tp, lt[:tp], "ln2_g")
            nc.scalar.dma_start(out=ln2_d[t0:t0 + tp], in_=lt[:tp])

        if stop_after <= 7:
            raise _StopBuild
        # ============ Stage 8+9: MLP (feature-major hidden) ============
        with tc.tile_pool(name="mlp_sb", bufs=1) as mlp_sb, \
             tc.tile_pool(name="m1_ps", bufs=2, space="PSUM") as ps1pool, \
             tc.tile_pool(name="m2_ps", bufs=3, space="PSUM") as ps2pool, \
             tc.tile_pool(name="m2_out", bufs=2) as opool:
            l2T = mlp_sb.tile([128, KD, T], BF16, tag="l2T")
            nc.scalar.dma_start(out=l2T[:], in_=ln2_d[:], transpose=True)
            if fp8:
                l2T8 = mlp_sb.tile([128, KD, T], WDT, tag="l2T8")
                nc.gpsimd.tensor_copy(out=l2T8[:], in_=l2T[:])
            else:
                l2T8 = l2T
            hT = mlp_sb.tile([128, KH, T], WDT, tag="hT")
            b1_fm = None
            if flags.get("b1"):
                b1_fm = mlp_sb.tile([128, KH], F32, tag="b1_fm")
                nc.sync.dma_start(
                    out=b1_fm, in_=opt_d["b1"][:].rearrange("(c p) -> p c", p=128)
                )
            # MLP1: out chunk mh (128 HID dims) = gelu(w1_chunk^T @ ln2^T)
            for ni, (n0, nw) in enumerate(_nsplit(HID)):
                wt = w1_tiles[ni]
                for j in range(4):
                    mh = ni * 4 + j
                    ps = ps1pool.tile([128, T], F32, tag="ps1")
                    for k in range(0, KD, KSTEP):
                        for s0, sw in _nsplit(T):
                            nc.tensor.matmul(
                                ps[:, s0:s0 + sw],
                                wt[:, k:k + KSTEP, j * 128:(j + 1) * 128],
                                l2T8[:, k:k + KSTEP, s0:s0 + sw],
                                start=(k == 0),
                                stop=(k == KD - KSTEP),
                                perf_mode=DRM,
                            )
                    bias = b1_fm[:, mh:mh + 1] if b1_fm is not None else 0.0
                    nc.scalar.activation(out=hT[:, mh], in_=ps[:], func=gelu,
                                         bias=bias, scale=descale)
            # MLP2: token-major out (w2 preloaded during attention)
            for ti, (t0, tp) in enumerate(TT):
                for n0, nw in _nsplit(D):
                    ps = ps2pool.tile([128, 512], F32, tag="ps2")
                    for k in range(0, KH, KSTEP):
                        nc.tensor.matmul(
                            ps[:tp],
                            hT[:, k:k + KSTEP, t0:t0 + tp],
                            wt2[:, k:k + KSTEP, n0:n0 + nw],
                            start=(k == 0),
                            stop=(k == KH - KSTEP),
                            perf_mode=DRM,
                        )
                    ot = opool.tile([128, 512], F32, tag="ot")
                    if fp8:
                        if "ls2s" in opt_d:
                            nc.scalar.activation(
                                out=ot[:tp], in_=ps[:tp], func=ACTF.Identity,
                                scale=ls_sb["ls2s"][:tp],
                            )
                        else:
                            nc.vector.tensor_mul(
                                ot[:tp], ps[:tp], ls_sb["ls2v"][:tp, n0:n0 + nw]
                            )
                        nc.vector.tensor_add(
                            ot[:tp], ot[:tp], r1[:tp, ti, n0:n0 + nw]
                        )
                    else:
                        nc.vector.tensor_add(
                            ot[:tp], ps[:tp], r1[:tp, ti, n0:n0 + nw]
                        )
                    if "b2" in rep:
                        nc.vector.tensor_add(
                            ot[:tp], ot[:tp], rep["b2"][:tp, n0:n0 + nw]
                        )
                    nc.scalar.dma_start(
                        out=out_d[t0:t0 + tp, n0:n0 + nw], in_=ot[:tp]
                    )


def _nontrivial(a, val):
    return not np.allclose(a, val, rtol=0.0, atol=0.0)


def prepare(inputs, fp8=True):
    """Host-side preprocessing: dtype casts and scale folding.

    Returns (flags, common_map) where common_map holds every device input
    except per-core "x".

    bf16 path: ls1/ls2 are folded into w_proj/w2.
    fp8 path: weights are scaled by W_SCALE (so sigma~0.32 stays in e4m3
    normal range; ls*1e-5 would underflow), and ls/W_SCALE is applied at
    eviction via the ls1s/ls2s (uniform) or ls1v/ls2v (vector) inputs.
    """
    f32 = np.float32
    bf16 = ml_dtypes.bfloat16
    w_qkv = np.asarray(inputs["w_qkv"], f32)
    w_proj = np.asarray(inputs["w_proj"], f32)
    w1 = np.asarray(inputs["w1"], f32)
    w2 = np.asarray(inputs["w2"], f32)
    ls1 = np.asarray(inputs["ls1"], f32)
    ls2 = np.asarray(inputs["ls2"], f32)

    flags = {
        "ln1": _nontrivial(inputs["ln1_g"], 1) or _nontrivial(inputs["ln1_b"], 0),
        "ln2": _nontrivial(inputs["ln2_g"], 1) or _nontrivial(inputs["ln2_b"], 0),
        "oln": _nontrivial(inputs["o_g"], 1) or _nontrivial(inputs["o_b"], 0),
        "qk": _nontrivial(inputs["q_g"], 1) or _nontrivial(inputs["q_b"], 0)
        or _nontrivial(inputs["k_g"], 1) or _nontrivial(inputs["k_b"], 0),
        "bproj": _nontrivial(inputs["b_proj"] * ls1, 0),
        "b1": _nontrivial(inputs["b1"], 0),
        "b2": _nontrivial(inputs["b2"] * ls2, 0),
    }

    if fp8:
        e4 = mybir.dt.np(FP8)
        flags["ls1u"] = bool(np.all(ls1 == ls1[0]))
        flags["ls2u"] = bool(np.all(ls2 == ls2[0]))
        cm = {
            "wqkv": (w_qkv * W_SCALE).astype(e4),
            "wproj": (w_proj * W_SCALE).astype(e4),
            "w1": (w1 * W_SCALE).astype(e4),
            "w2": (w2 * W_SCALE).astype(e4),
        }
        if flags["ls1u"]:
            cm["ls1s"] = (ls1[:1] / W_SCALE).astype(f32)
        else:
            cm["ls1v"] = (ls1 / W_SCALE).astype(f32)
        if flags["ls2u"]:
            cm["ls2s"] = (ls2[:1] / W_SCALE).astype(f32)
        else:
            cm["ls2v"] = (ls2 / W_SCALE).astype(f32)
    else:
        cm = {
            "wqkv": w_qkv.astype(bf16),
            "wproj": (w_proj * ls1[None, :]).astype(bf16),
            "w1": w1.astype(bf16),
            "w2": (w2 * ls2[None, :]).astype(bf16),
        }
    if flags["ln1"]:
        cm["ln1_g"] = np.asarray(inputs["ln1_g"], f32)
        cm["ln1_b"] = np.asarray(inputs["ln1_b"], f32)
    if flags["ln2"]:
        cm["ln2_g"] = np.asarray(inputs["ln2_g"], f32)
        cm["ln2_b"] = np.asarray(inputs["ln2_b"], f32)
    if flags["oln"]:
        cm["o_g"] = np.asarray(inputs["o_g"], f32)
        cm["o_b"] = np.asarray(inputs["o_b"], f32)
    if flags["qk"]:
        cm["qg"] = np.asarray(inputs["q_g"], f32)
        cm["qb"] = np.asarray(inputs["q_b"], f32)
        cm["kg"] = np.asarray(inputs["k_g"], f32)
        cm["kb"] = np.asarray(inputs["k_b"], f32)
    if flags["bproj"]:
        cm["bproj"] = (np.asarray(inputs["b_proj"], f32) * ls1).astype(f32)
    if flags["b1"]:
        cm["b1"] = np.asarray(inputs["b1"], f32)
    if flags["b2"]:
        cm["b2"] = (np.asarray(inputs["b2"], f32) * ls2).astype(f32)
    return flags, cm


_CACHE = {}


def get_compiled(flags, fp8=True):
    key = (fp8,) + tuple(sorted((k, bool(v)) for k, v in flags.items()))
    if key not in _CACHE:
        _CACHE[key] = build_block(T=576, flags=flags, fp8=fp8)
    return _CACHE[key]


def kernel(**inputs):
    from concourse import bass_utils

    x = np.asarray(inputs["x"], np.float32)
    B = x.shape[0]
    assert B == N_CORES
    fp8 = True
    flags, cm = prepare(inputs, fp8=fp8)
    nc = get_compiled(flags, fp8=fp8)
    in_maps = [dict(cm, x=np.ascontiguousarray(x[i])) for i in range(B)]
    res = bass_utils.run_bass_kernel_spmd(nc, in_maps, core_ids=list(range(B)))
    out = np.stack([res.results[i]["out"] for i in range(B)], axis=0)
    return out.astype(np.float32)


if __name__ == "__main__":
    import reference

    inputs = {k: np.asarray(v) for k, v in reference.setup_inputs().items()}
    expected = np.asarray(reference.reference(**reference.setup_inputs()))
    actual = kernel(**inputs)
    err = np.linalg.norm(actual - expected) / np.linalg.norm(expected)
    print("Relative error:", err)


# revision 47
# speedup vs baseline: 1.7861x; 1.1243x over previous
"""Trainium2 Bass kernel for a dense transformer block.

Shapes (hardcoded from the problem spec):
  x [B=8, P=576, D=1024], H=16 heads, HD=64, HID=4096.

Sharding: data-parallel over batch. Core i processes batch element i
(576 tokens); weights are replicated to every core; no collectives.

Per-core dataflow (token-major activations, bf16 matmuls, f32 residuals):
  LN1 -> QKV matmul -> QK-LayerNorm -> per-head attention
  (S^T = k @ q^T via PE, exp on ACT without max-subtraction -- QK-LN
  bounds |logits| <= 8 -- then O = exp(S^T)^T @ [v|ones] which yields the
  softmax numerator and denominator in a single matmul) -> o-LN -> proj
  (+residual; ls1 folded into w_proj on host) -> LN2 -> MLP1 emitted
  feature-major (so GELU output IS the transposed input of MLP2, no
  DRAM bounce; b1 becomes a per-partition ACT bias) -> MLP2 -> residual.

Token-major <-> feature-major transposes bounce bf16 tensors through
DRAM and re-load with single XBAR transpose-DMA instructions (3D out).
"""

import sys

if "/opt/trn_rl_repo" not in sys.path:
    sys.path.insert(0, "/opt/trn_rl_repo")

import math
from contextlib import ExitStack

import ml_dtypes
import numpy as np

import concourse.bass as bass
import concourse.bacc as bacc
import concourse.mybir as mybir
import concourse.tile as tile
from concourse.masks import make_identity

F32 = mybir.dt.float32
BF16 = mybir.dt.bfloat16
FP8 = mybir.dt.float8e4
W_SCALE = 16.0  # host multiplies fp8 weights by this; descaled at eviction
AX = mybir.AxisListType.X
OP = mybir.AluOpType
ACTF = mybir.ActivationFunctionType

D = 1024
H = 16
HD = 64
HID = 4096
EPS = 1e-6
N_CORES = 8


def _ttiles(T):
    return [(i * 128, min(128, T - i * 128)) for i in range(math.ceil(T / 128))]


def _nsplit(n, chunk=512):
    out = []
    o = 0
    while o < n:
        out.append((o, min(chunk, n - o)))
        o += chunk
    return out


class _StopBuild(Exception):
    pass


def build_block(T=576, flags=None, gelu=ACTF.Gelu_apprx_tanh, fp8=True, dr=True):
    """Builds the Bass program for one core (T tokens). Returns nc.

    flags: dict of booleans for which optional vector inputs are
    non-trivial and must be applied on-device:
      ln1, ln2, oln (gamma/beta), qk (q/k gamma/beta), bproj, b1, b2
    """
    flags = flags or {}
    nc = bacc.Bacc("TRN2", target_bir_lowering=False, debug=False)
    fp8 = bool(fp8)

    TT = _ttiles(T)
    NT = len(TT)
    KD = D // 128  # 8 k-chunks over D
    KH = HID // 128  # 32 k-chunks over HID

    # ---------------- DRAM I/O ----------------
    x_d = nc.dram_tensor("x", [T, D], F32, kind="ExternalInput")
    wdt = FP8 if fp8 else BF16
    wqkv_d = nc.dram_tensor("wqkv", [D, 3 * D], wdt, kind="ExternalInput")
    wproj_d = nc.dram_tensor("wproj", [D, D], wdt, kind="ExternalInput")
    w1_d = nc.dram_tensor("w1", [D, HID], wdt, kind="ExternalInput")
    w2_d = nc.dram_tensor("w2", [HID, D], wdt, kind="ExternalInput")
    out_d = nc.dram_tensor("out", [T, D], F32, kind="ExternalOutput")

    opt_d = {}
    for name, shape, want in [
        ("ln1_g", [D], flags.get("ln1")),
        ("ln1_b", [D], flags.get("ln1")),
        ("ln2_g", [D], flags.get("ln2")),
        ("ln2_b", [D], flags.get("ln2")),
        ("o_g", [D], flags.get("oln")),
        ("o_b", [D], flags.get("oln")),
        ("qg", [HD], flags.get("qk")),
        ("qb", [HD], flags.get("qk")),
        ("kg", [HD], flags.get("qk")),
        ("kb", [HD], flags.get("qk")),
        ("bproj", [D], flags.get("bproj")),
        ("b1", [HID], flags.get("b1")),
        ("b2", [D], flags.get("b2")),
        ("ls1s", [1], fp8 and flags.get("ls1u", True)),
        ("ls2s", [1], fp8 and flags.get("ls2u", True)),
        ("ls1v", [D], fp8 and not flags.get("ls1u", True)),
        ("ls2v", [D], fp8 and not flags.get("ls2u", True)),
    ]:
        if want:
            opt_d[name] = nc.dram_tensor(name, shape, F32, kind="ExternalInput")

    try:
        _build_body(nc, T, flags, gelu, x_d, wqkv_d, wproj_d, w1_d, w2_d,
                    out_d, opt_d, fp8, dr)
    except _StopBuild:
        pass
    nc.compile()
    return nc


def _build_body(nc, T, flags, gelu, x_d, wqkv_d, wproj_d, w1_d, w2_d,
                out_d, opt_d, fp8, dr):
    WDT = FP8 if fp8 else BF16
    DRM = mybir.MatmulPerfMode.DoubleRow if (fp8 and dr) else None
    KSTEP = 2 if (fp8 and dr) else 1
    descale = (1.0 / W_SCALE) if fp8 else 1.0
    TT = _ttiles(T)
    NT = len(TT)
    KD = D // 128
    KH = HID // 128
    with tile.TileContext(nc) as tc, ExitStack() as ctx:
        # ---------------- pools ----------------
        persist = ctx.enter_context(tc.tile_pool(name="persist", bufs=1))
        dram = ctx.enter_context(tc.tile_pool(name="dram", bufs=1, space="DRAM"))
        stats = ctx.enter_context(tc.tile_pool(name="stats", bufs=4))
        ev = ctx.enter_context(tc.tile_pool(name="ev", bufs=2))

        # constants
        eps_t = persist.tile([128, 1], F32, tag="eps")
        nc.vector.memset(eps_t, EPS)

        # replicated optional vectors (broadcast over partitions)
        rep = {}
        for name, width in [
            ("ln1_g", D), ("ln1_b", D), ("ln2_g", D), ("ln2_b", D),
            ("o_g", D), ("o_b", D), ("qg", HD), ("qb", HD),
            ("kg", HD), ("kb", HD), ("bproj", D), ("b2", D),
        ]:
            if name in opt_d:
                t = persist.tile([128, width], F32, tag=f"rep_{name}")
                nc.sync.dma_start(out=t, in_=opt_d[name][:].to_broadcast([128, width]))
                rep[name] = t

        # per-partition ls1/ls2 descale scalars (fp8 path)
        ls_sb = {}
        for nm in ("ls1s", "ls2s"):
            if nm in opt_d:
                t = persist.tile([128, 1], F32, tag=f"ls_{nm}")
                nc.sync.dma_start(out=t, in_=opt_d[nm][:].to_broadcast([128, 1]))
                ls_sb[nm] = t
        for nm in ("ls1v", "ls2v"):
            if nm in opt_d:
                t = persist.tile([128, D], F32, tag=f"ls_{nm}")
                nc.sync.dma_start(out=t, in_=opt_d[nm][:].to_broadcast([128, D]))
                ls_sb[nm] = t

        r1 = persist.tile([128, NT, D], F32, tag="r1")
        l2T8 = persist.tile([128, KD, T], WDT, tag="l2T8")
        ident = persist.tile([128, 128], BF16, tag="ident")
        make_identity(nc, ident)
        def pe_transpose_tile(tr_ps, src_tile, tp, dst, t0, name):
            """dst[:, :, t0:t0+tp] = blockwise transpose of src [tp, KD*128].

            All KD 128-col blocks transpose into one bf16 psum bank, then
            one eviction copies (and casts) into the [128, KD, T] operand.
            """
            ps = tr_ps.tile([128, KD, 128], BF16, tag="tr", name=name)
            for c in range(KD):
                nc.tensor.transpose(
                    ps[:, c, :tp],
                    src_tile[:tp, c * 128:(c + 1) * 128],
                    ident[:tp, :tp],
                )
            nc.any.tensor_copy(out=dst[:, :, t0:t0 + tp], in_=ps[:, :, :tp])

        # ---------- helper: token-major layernorm over D ----------
        def ln_tile(src_ap, tp, dst_bf16, gname):
            """dst = LN(src) (* g + b if flagged). src [tp, D] f32/bf16."""
            st = stats.tile([128, 2, nc.vector.BN_STATS_DIM], F32, tag="ln_st", bufs=2)
            for s in range(2):
                nc.vector.bn_stats(
                    out=st[:tp, s], in_=src_ap[:, s * 512:(s + 1) * 512]
                )
            mv = stats.tile([128, nc.vector.BN_AGGR_DIM], F32, tag="ln_mv")
            nc.vector.bn_aggr(out=mv[:tp], in_=st[:tp])
            sd = stats.tile([128, 1], F32, tag="ln_sd")
            nc.scalar.activation(
                out=sd[:tp], in_=mv[:tp, 1:2], func=ACTF.Sqrt, bias=eps_t[:tp]
            )
            rstd = stats.tile([128, 1], F32, tag="ln_rstd")
            nc.vector.reciprocal(out=rstd[:tp], in_=sd[:tp])
            negmr = stats.tile([128, 1], F32, tag="ln_negmr")
            nc.vector.tensor_tensor(
                negmr[:tp], mv[:tp, 0:1], rstd[:tp], OP.mult
            )
            nc.vector.tensor_scalar_mul(negmr[:tp], negmr[:tp], -1.0)
            nc.scalar.activation(
                out=dst_bf16,
                in_=src_ap,
                func=ACTF.Identity,
                bias=negmr[:tp],
                scale=rstd[:tp],
            )
            if gname in rep:
                nc.vector.tensor_mul(dst_bf16, dst_bf16, rep[gname][:tp])
                nc.vector.tensor_add(
                    dst_bf16, dst_bf16, rep[gname.replace("_g", "_b")][:tp]
                )

        stop_after = flags.get("stop_after", 99)
        with tc.tile_pool(name="blk1", bufs=1) as blk1:
            tr_a_cm = tc.tile_pool(name="tr_a", bufs=2, space="PSUM",
                                   side="right")
            tr_a = tr_a_cm.__enter__()
            xres = blk1.tile([128, NT, D], F32, tag="xres")
            xT8 = blk1.tile([128, KD, T], WDT, tag="xT8")
            oT8 = blk1.tile([128, KD, T], WDT, tag="oT8")
            qkT_cm = tc.tile_pool(name="att_qkT", bufs=1)
            qkT_pool = qkT_cm.__enter__()
            qT = qkT_pool.tile([128, KD, T], BF16, tag="qT")
            kT = qkT_pool.tile([128, KD, T], BF16, tag="kT")
            qkv_cm = tc.tile_pool(name="qkv_sb", bufs=1)
            qkv_pool = qkv_cm.__enter__()
            qkv = qkv_pool.tile([128, NT, 3 * D], BF16, tag="qkv")
            v_aug = blk1.tile([128, NT, H, HD + 1], BF16, tag="v_aug")
            attn = blk1.tile([128, NT, D], BF16, tag="attn")
            rk_sb = blk1.tile([128, NT, H], F32, tag="rk_sb")

            # ================= Stage 1: load x, LN1 =================
            for ti, (t0, tp) in enumerate(TT):
                nc.sync.dma_start(out=xres[:tp, ti], in_=x_d[t0:t0 + tp])
            for ti, (t0, tp) in enumerate(TT):
                xln_t = ev.tile([128, D], BF16, tag="xln_t")
                ln_tile(xres[:tp, ti], tp, xln_t[:tp], "ln1_g")
                pe_transpose_tile(tr_a, xln_t, tp, xT8, t0, f"trx{ti}")

            if stop_after <= 1:
                raise _StopBuild
            # ================= Stage 2: QKV matmul =================
            with tc.tile_pool(name="qkv_w", bufs=6) as wpool, \
                 tc.tile_pool(name="qkv_ps", bufs=4, space="PSUM") as pspool:
                wq_r = wqkv_d[:].rearrange("(ko p) n -> p ko n", p=128)
                for n0, nw in _nsplit(3 * D):
                    wt = wpool.tile([128, KD, 512], WDT, tag="wt")
                    nc.sync.dma_start(out=wt[:, :, :nw], in_=wq_r[:, :, n0:n0 + nw])
                    for ti, (t0, tp) in enumerate(TT):
                        ps = pspool.tile([128, 512], F32, tag="ps")
                        for k in range(0, KD, KSTEP):
                            nc.tensor.matmul(
                                ps[:tp, :nw],
                                xT8[:, k:k + KSTEP, t0:t0 + tp],
                                wt[:, k:k + KSTEP, :nw],
                                start=(k == 0),
                                stop=(k == KD - KSTEP),
                                perf_mode=DRM,
                            )
                        if fp8:
                            nc.any.tensor_scalar_mul(
                                qkv[:tp, ti, n0:n0 + nw], ps[:tp, :nw], descale
                            )
                        else:
                            nc.any.tensor_copy(
                                out=qkv[:tp, ti, n0:n0 + nw], in_=ps[:tp, :nw]
                            )

            if stop_after <= 2:
                raise _StopBuild
            # ================= Stage 3: QK-LN, build v_aug =================
            inv_hd = 1.0 / HD
            scale = HD ** (-0.5)
            for ti, (t0, tp) in enumerate(TT):
                # v | ones
                nc.vector.memset(v_aug[:tp, ti, :, HD:], 1.0)
                if tp < 128:
                    nc.vector.memset(v_aug[tp:, ti, :, HD:], 0.0)
                nc.gpsimd.tensor_copy(
                    out=v_aug[:tp, ti, :, :HD],
                    in_=qkv[:tp, ti, 2 * D:3 * D].rearrange("p (h d) -> p h d", h=H),
                )
                qk_fast = "qg" not in rep
                for which, base, gkey in (
                    ("q", 0, "qg"),
                    ("k", D, "kg"),
                ):
                    src = qkv[:tp, ti, base:base + D].rearrange(
                        "p (h d) -> p h d", h=H
                    )
                    sq = stats.tile([128, H, HD], F32, tag="qk_sq", bufs=1)
                    nc.gpsimd.tensor_mul(sq[:tp], src, src)
                    s1 = stats.tile([128, H], F32, tag="qk_s1")
                    nc.vector.reduce_sum(out=s1[:tp], in_=src, axis=AX)
                    s2 = stats.tile([128, H], F32, tag="qk_s2")
                    nc.vector.reduce_sum(out=s2[:tp], in_=sq[:tp], axis=AX)
                    mean = stats.tile([128, H], F32, tag="qk_mean")
                    nc.vector.tensor_scalar_mul(mean[:tp], s1[:tp], inv_hd)
                    msq = stats.tile([128, H], F32, tag="qk_msq")
                    nc.vector.tensor_mul(msq[:tp], mean[:tp], mean[:tp])
                    var = stats.tile([128, H], F32, tag="qk_var")
                    nc.vector.tensor_scalar(
                        out=var[:tp], in0=s2[:tp], scalar1=inv_hd, scalar2=None,
                        op0=OP.mult,
                    )
                    nc.vector.tensor_sub(var[:tp], var[:tp], msq[:tp])
                    sd = stats.tile([128, H], F32, tag="qk_sd")
                    nc.scalar.activation(
                        out=sd[:tp], in_=var[:tp], func=ACTF.Sqrt, bias=eps_t[:tp]
                    )
                    lnt = ev.tile([128, H, HD], BF16, tag="qk_out")
                    if qk_fast:
                        # k is centered only (rk folded into exp scale);
                        # q is scaled by rstd*hd^-0.5 only (its mean term
                        # vanishes against centered k).
                        if which == "k":
                            nc.vector.reciprocal(
                                out=rk_sb[:tp, ti], in_=sd[:tp]
                            )
                            nc.gpsimd.tensor_tensor(
                                lnt[:tp], src,
                                mean[:tp, :, None].to_broadcast([tp, H, HD]),
                                OP.subtract,
                            )
                        else:
                            rq = stats.tile([128, H], F32, tag="qk_rq")
                            nc.vector.reciprocal(out=rq[:tp], in_=sd[:tp])
                            nc.vector.tensor_scalar_mul(rq[:tp], rq[:tp], scale)
                            nc.vector.tensor_tensor(
                                lnt[:tp], src,
                                rq[:tp, :, None].to_broadcast([tp, H, HD]),
                                OP.mult,
                            )
                    else:
                        rstd = stats.tile([128, H], F32, tag="qk_rstd")
                        nc.vector.reciprocal(out=rstd[:tp], in_=sd[:tp])
                        nc.gpsimd.tensor_tensor(
                            lnt[:tp], src,
                            mean[:tp, :, None].to_broadcast([tp, H, HD]),
                            OP.subtract,
                        )
                        nc.vector.tensor_tensor(
                            lnt[:tp], lnt[:tp],
                            rstd[:tp, :, None].to_broadcast([tp, H, HD]), OP.mult,
                        )
                        g = rep[gkey]
                        b = rep["qb" if which == "q" else "kb"]
                        nc.vector.tensor_tensor(
                            lnt[:tp], lnt[:tp],
                            g[:tp, None, :].to_broadcast([tp, H, HD]), OP.mult,
                        )
                        nc.vector.tensor_tensor(
                            lnt[:tp], lnt[:tp],
                            b[:tp, None, :].to_broadcast([tp, H, HD]), OP.add,
                        )
                        if which == "q":
                            nc.vector.tensor_scalar_mul(lnt[:tp], lnt[:tp], scale)
                    flat = lnt[:tp].rearrange("p h d -> p (h d)")
                    dstT = qT if which == "q" else kT
                    pe_transpose_tile(tr_a, flat, tp, dstT, t0, f"tr{which}{ti}")

            # qkv dead; free its SBUF and start MLP weight streams into it
            qkv_cm.__exit__(None, None, None)
            mlp_w = ctx.enter_context(
                tc.tile_pool(name="mlp_w", bufs=1, side="right")
            )
            w1_r = w1_d[:].rearrange("(ko p) n -> p ko n", p=128)
            w1_tiles = []
            for ni, (n0, nw) in enumerate(_nsplit(HID)):
                wt = mlp_w.tile([128, KD, 512], WDT, tag=f"w1_{ni}",
                                name=f"w1t{ni}")
                nc.sync.dma_start(out=wt[:], in_=w1_r[:, :, n0:n0 + nw])
                w1_tiles.append(wt)
            wt2 = mlp_w.tile([128, KH, D], WDT, tag="wt2")
            nc.sync.dma_start(
                out=wt2[:], in_=w2_d[:].rearrange("(ko p) n -> p ko n", p=128)
            )

            tr_a_cm.__exit__(None, None, None)

            # ================= Stage 4: attention =================
            with tc.tile_pool(name="att_exp", bufs=2) as exp_pool, \
                 tc.tile_pool(name="att_ps", bufs=3, space="PSUM") as qk_ps_pool, \
                 tc.tile_pool(name="av_ps", bufs=2, space="PSUM") as av_ps_pool:
                def qk_exp(h):
                    c, off = h // 2, (h % 2) * 64
                    q_h = qT[off:off + 64, c]
                    k_h = kT[off:off + 64, c]
                    exp_tiles = []
                    for tk, (tk0, tkw) in enumerate(TT):
                        ps = qk_ps_pool.tile(
                            [128, T], F32, tag="qk_ps", name=f"qkps{h}_{tk}"
                        )
                        for n0, nw in _nsplit(T):
                            nc.tensor.matmul(
                                ps[:tkw, n0:n0 + nw],
                                k_h[:, tk0:tk0 + tkw],
                                q_h[:, n0:n0 + nw],
                                start=True,
                                stop=True,
                            )
                        et = exp_pool.tile(
                            [128, T], BF16, tag=f"exp{tk}", name=f"exp{h}_{tk}"
                        )
                        if "qg" not in rep:
                            nc.scalar.activation(
                                out=et[:tkw], in_=ps[:tkw], func=ACTF.Exp,
                                scale=rk_sb[:tkw, tk, h:h + 1],
                            )
                        else:
                            nc.scalar.activation(
                                out=et[:tkw], in_=ps[:tkw], func=ACTF.Exp
                            )
                        exp_tiles.append(et)
                    return exp_tiles

                def av(h, exp_tiles):
                    for mi, (m0, mp) in enumerate(TT):
                        pso = av_ps_pool.tile(
                            [128, HD + 1], F32, tag="av_ps", name=f"avps{h}_{mi}"
                        )
                        for tk, (tk0, tkw) in enumerate(TT):
                            nc.tensor.matmul(
                                pso[:mp],
                                exp_tiles[tk][:tkw, m0:m0 + mp],
                                v_aug[:tkw, tk, h],
                                start=(tk == 0),
                                stop=(tk == NT - 1),
                            )
                        rc = stats.tile([128, 1], F32, tag="att_rc")
                        nc.vector.reciprocal(out=rc[:mp], in_=pso[:mp, HD:])
                        nc.vector.tensor_scalar_mul(
                            attn[:mp, mi, h * HD:(h + 1) * HD],
                            pso[:mp, :HD], rc[:mp],
                        )

                prev = None
                for h in range(H):
                    cur = qk_exp(h)
                    if prev is not None:
                        av(h - 1, prev)
                    prev = cur
                av(H - 1, prev)
            qkT_cm.__exit__(None, None, None)

            tr_b = ctx.enter_context(
                tc.tile_pool(name="tr_b", bufs=2, space="PSUM", side="right")
            )
            # ================= Stage 5: o-LN =================
            for ti, (t0, tp) in enumerate(TT):
                ot = ev.tile([128, D], BF16, tag="oln_t")
                ln_tile(attn[:tp, ti], tp, ot[:tp], "o_g")
                pe_transpose_tile(tr_b, ot, tp, oT8, t0, f"tro{ti}")

            if stop_after <= 5:
                raise _StopBuild
            # ================= Stage 6: proj + residual =================
            with tc.tile_pool(name="proj_w", bufs=2) as wpool, \
                 tc.tile_pool(name="proj_ps", bufs=4, space="PSUM") as pspool:
                wp_r = wproj_d[:].rearrange("(ko p) n -> p ko n", p=128)
                for n0, nw in _nsplit(D):
                    wt = wpool.tile([128, KD, 512], WDT, tag="wt")
                    nc.sync.dma_start(out=wt[:, :, :nw], in_=wp_r[:, :, n0:n0 + nw])
                    for ti, (t0, tp) in enumerate(TT):
                        ps = pspool.tile([128, 512], F32, tag="ps")
                        for k in range(0, KD, KSTEP):
                            nc.tensor.matmul(
                                ps[:tp, :nw],
                                oT8[:, k:k + KSTEP, t0:t0 + tp],
                                wt[:, k:k + KSTEP, :nw],
                                start=(k == 0),
                                stop=(k == KD - KSTEP),
                                perf_mode=DRM,
                            )
                        dst = r1[:tp, ti, n0:n0 + nw]
                        if fp8:
                            if "ls1s" in opt_d:
                                nc.scalar.activation(
                                    out=dst, in_=ps[:tp, :nw],
                                    func=ACTF.Identity,
                                    scale=ls_sb["ls1s"][:tp],
                                )
                            else:
                                nc.vector.tensor_mul(
                                    dst, ps[:tp, :nw],
                                    ls_sb["ls1v"][:tp, n0:n0 + nw],
                                )
                            nc.vector.tensor_add(
                                dst, dst, xres[:tp, ti, n0:n0 + nw]
                            )
                        else:
                            nc.vector.tensor_add(
                                dst, ps[:tp, :nw], xres[:tp, ti, n0:n0 + nw]
                            )
                        if "bproj" in rep:
                            nc.vector.tensor_add(
                                dst, dst, rep["bproj"][:tp, n0:n0 + nw]
                            )

        if stop_after <= 6:
            raise _StopBuild
        # ================= Stage 7: LN2 =================
        for ti, (t0, tp) in enumerate(TT):
            lt = ev.tile([128, D], BF16, tag="ln2_t")
            ln_tile(r1[:tp, ti], tp, lt[:tp], "ln2_g")
            pe_transpose_tile(tr_b, lt, tp, l2T8, t0, f"trl{ti}")

        if stop_after <= 7:
            raise _StopBuild
        # ============ Stage 8+9: MLP (feature-major hidden) ============
        with tc.tile_pool(name="mlp_sb", bufs=1) as mlp_sb, \
             tc.tile_pool(name="m1_ps", bufs=2, space="PSUM") as ps1pool, \
             tc.tile_pool(name="m2_ps", bufs=2, space="PSUM") as ps2pool, \
             tc.tile_pool(name="m2_out", bufs=2) as opool:
            hT = mlp_sb.tile([128, KH, T], WDT, tag="hT")
            b1_fm = None
            if flags.get("b1"):
                b1_fm = mlp_sb.tile([128, KH], F32, tag="b1_fm")
                nc.sync.dma_start(
                    out=b1_fm, in_=opt_d["b1"][:].rearrange("(c p) -> p c", p=128)
                )
            # MLP1: out chunk mh (128 HID dims) = gelu(w1_chunk^T @ ln2^T)
            for ni, (n0, nw) in enumerate(_nsplit(HID)):
                wt = w1_tiles[ni]
                for j in range(4):
                    mh = ni * 4 + j
                    ps = ps1pool.tile([128, T], F32, tag="ps1")
                    for k in range(0, KD, KSTEP):
                        for s0, sw in _nsplit(T):
                            nc.tensor.matmul(
                                ps[:, s0:s0 + sw],
                                wt[:, k:k + KSTEP, j * 128:(j + 1) * 128],
                                l2T8[:, k:k + KSTEP, s0:s0 + sw],
                                start=(k == 0),
                                stop=(k == KD - KSTEP),
                                perf_mode=DRM,
                            )
                    bias = b1_fm[:, mh:mh + 1] if b1_fm is not None else 0.0
                    nc.scalar.activation(out=hT[:, mh], in_=ps[:], func=gelu,
                                         bias=bias, scale=descale)
            # MLP2 pass A (k 0..15) overlaps the tail of MLP1; partial
            # sums parked in SBUF. Pass B (k 16..31) adds them at eviction.
            KHH = KH // 2
            part = mlp_sb.tile([128, NT, D], F32, tag="m2part")
            for ti, (t0, tp) in enumerate(TT):
                for n0, nw in _nsplit(D):
                    ps = ps2pool.tile([128, 512], F32, tag="ps2",
                                      name=f"m2a{ti}_{n0}")
                    for k in range(0, KHH, KSTEP):
                        nc.tensor.matmul(
                            ps[:tp],
                            hT[:, k:k + KSTEP, t0:t0 + tp],
                            wt2[:, k:k + KSTEP, n0:n0 + nw],
                            start=(k == 0),
                            stop=(k == KHH - KSTEP),
                            perf_mode=DRM,
                        )
                    nc.any.tensor_copy(
                        out=part[:tp, ti, n0:n0 + nw], in_=ps[:tp]
                    )
            for ti, (t0, tp) in enumerate(TT):
                for n0, nw in _nsplit(D):
                    ps = ps2pool.tile([128, 512], F32, tag="ps2",
                                      name=f"m2b{ti}_{n0}")
                    for k in range(KHH, KH, KSTEP):
                        nc.tensor.matmul(
                            ps[:tp],
                            hT[:, k:k + KSTEP, t0:t0 + tp],
                            wt2[:, k:k + KSTEP, n0:n0 + nw],
                            start=(k == KHH),
                            stop=(k == KH - KSTEP),
                            perf_mode=DRM,
                        )
                    ot = opool.tile([128, 512], F32, tag="ot")
                    nc.vector.tensor_add(
                        ot[:tp], ps[:tp], part[:tp, ti, n0:n0 + nw]
                    )
                    if fp8:
                        if "ls2s" in opt_d:
                            nc.scalar.activation(
                                out=ot[:tp], in_=ot[:tp], func=ACTF.Identity,
                                scale=ls_sb["ls2s"][:tp],
                            )
                        else:
                            nc.vector.tensor_mul(
                                ot[:tp], ot[:tp], ls_sb["ls2v"][:tp, n0:n0 + nw]
                            )
                    nc.vector.tensor_add(
                        ot[:tp], ot[:tp], r1[:tp, ti, n0:n0 + nw]
                    )
                    if "b2" in rep:
                        nc.vector.tensor_add(
                            ot[:tp], ot[:tp], rep["b2"][:tp, n0:n0 + nw]
                        )
                    nc.scalar.dma_start(
                        out=out_d[t0:t0 + tp, n0:n0 + nw], in_=ot[:tp]
                    )

    nc.compile()
    return nc


def _build_body(nc, T, flags, gelu, x_d, wqkv_d, wproj_d, w1_d, w2_d,
                out_d, opt_d, fp8, dr):
    WDT = FP8 if fp8 else BF16
    DRM = mybir.MatmulPerfMode.DoubleRow if (fp8 and dr) else None
    KSTEP = 2 if (fp8 and dr) else 1
    descale = (1.0 / W_SCALE) if fp8 else 1.0
    TT = _ttiles(T)
    NT = len(TT)
    KD = D // 128
    KH = HID // 128
    with tile.TileContext(nc) as tc, ExitStack() as ctx:
        # ---------------- pools ----------------
        persist = ctx.enter_context(tc.tile_pool(name="persist", bufs=1))
        dram = ctx.enter_context(tc.tile_pool(name="dram", bufs=1, space="DRAM"))
        stats = ctx.enter_context(tc.tile_pool(name="stats", bufs=4))
        ev = ctx.enter_context(tc.tile_pool(name="ev", bufs=2))

        # constants
        eps_t = persist.tile([128, 1], F32, tag="eps")
        nc.vector.memset(eps_t, EPS)

        # replicated optional vectors (broadcast over partitions)
        rep = {}
        for name, width in [
            ("ln1_g", D), ("ln1_b", D), ("ln2_g", D), ("ln2_b", D),
            ("o_g", D), ("o_b", D), ("qg", HD), ("qb", HD),
            ("kg", HD), ("kb", HD), ("bproj", D), ("b2", D),
        ]:
            if name in opt_d:
                t = persist.tile([128, width], F32, tag=f"rep_{name}")
                nc.sync.dma_start(out=t, in_=opt_d[name][:].to_broadcast([128, width]))
                rep[name] = t

        # per-partition ls1/ls2 descale scalars (fp8 path)
        ls_sb = {}
        for nm in ("ls1s", "ls2s"):
            if nm in opt_d:
                t = persist.tile([128, 1], F32, tag=f"ls_{nm}")
                nc.sync.dma_start(out=t, in_=opt_d[nm][:].to_broadcast([128, 1]))
                ls_sb[nm] = t
        for nm in ("ls1v", "ls2v"):
            if nm in opt_d:
                t = persist.tile([128, D], F32, tag=f"ls_{nm}")
                nc.sync.dma_start(out=t, in_=opt_d[nm][:].to_broadcast([128, D]))
                ls_sb[nm] = t

        r1 = persist.tile([128, NT, D], F32, tag="r1")
        l2T8 = persist.tile([128, KD, T], WDT, tag="l2T8")
        ident = persist.tile([128, 128], BF16, tag="ident")
        make_identity(nc, ident)
        def pe_transpose_tile(tr_ps, src_tile, tp, dst, t0, name):
            """dst[:, :, t0:t0+tp] = blockwise transpose of src [tp, KD*128].

            All KD 128-col blocks transpose into one bf16 psum bank, then
            one eviction copies (and casts) into the [128, KD, T] operand.
            """
            ps = tr_ps.tile([128, KD, 128], BF16, tag="tr", name=name)
            for c in range(KD):
                nc.tensor.transpose(
                    ps[:, c, :tp],
                    src_tile[:tp, c * 128:(c + 1) * 128],
                    ident[:tp, :tp],
                )
            nc.any.tensor_copy(out=dst[:, :, t0:t0 + tp], in_=ps[:, :, :tp])

        # ---------- helper: token-major layernorm over D ----------
        def ln_tile(src_ap, tp, dst_bf16, gname):
            """dst = LN(src) (* g + b if flagged). src [tp, D] f32/bf16."""
            st = stats.tile([128, 2, nc.vector.BN_STATS_DIM], F32, tag="ln_st", bufs=2)
            for s in range(2):
                nc.vector.bn_stats(
                    out=st[:tp, s], in_=src_ap[:, s * 512:(s + 1) * 512]
                )
            mv = stats.tile([128, nc.vector.BN_AGGR_DIM], F32, tag="ln_mv")
            nc.vector.bn_aggr(out=mv[:tp], in_=st[:tp])
            sd = stats.tile([128, 1], F32, tag="ln_sd")
            nc.scalar.activation(
                out=sd[:tp], in_=mv[:tp, 1:2], func=ACTF.Sqrt, bias=eps_t[:tp]
            )
            rstd = stats.tile([128, 1], F32, tag="ln_rstd")
            nc.vector.reciprocal(out=rstd[:tp], in_=sd[:tp])
            negmr = stats.tile([128, 1], F32, tag="ln_negmr")
            nc.vector.tensor_tensor(
                negmr[:tp], mv[:tp, 0:1], rstd[:tp], OP.mult
            )
            nc.vector.tensor_scalar_mul(negmr[:tp], negmr[:tp], -1.0)
            nc.scalar.activation(
                out=dst_bf16,
                in_=src_ap,
                func=ACTF.Identity,
                bias=negmr[:tp],
                scale=rstd[:tp],
            )
            if gname in rep:
                nc.vector.tensor_mul(dst_bf16, dst_bf16, rep[gname][:tp])
                nc.vector.tensor_add(
                    dst_bf16, dst_bf16, rep[gname.replace("_g", "_b")][:tp]
                )

        stop_after = flags.get("stop_after", 99)
        with tc.tile_pool(name="blk1", bufs=1) as blk1:
            tr_a_cm = tc.tile_pool(name="tr_a", bufs=2, space="PSUM",
                                   side="right")
            tr_a = tr_a_cm.__enter__()
            xres = blk1.tile([128, NT, D], F32, tag="xres")
            xT8 = blk1.tile([128, KD, T], WDT, tag="xT8")
            oT8 = blk1.tile([128, KD, T], WDT, tag="oT8")
            qkT_cm = tc.tile_pool(name="att_qkT", bufs=1)
            qkT_pool = qkT_cm.__enter__()
            qT = qkT_pool.tile([128, KD, T], BF16, tag="qT")
            kT = qkT_pool.tile([128, KD, T], BF16, tag="kT")
            qkv_cm = tc.tile_pool(name="qkv_sb", bufs=1)
            qkv_pool = qkv_cm.__enter__()
            qkv = qkv_pool.tile([128, NT, 3 * D], BF16, tag="qkv")
            v_aug = blk1.tile([128, NT, H, HD + 1], BF16, tag="v_aug")
            attn = blk1.tile([128, NT, D], BF16, tag="attn")
            rk_sb = blk1.tile([128, NT, H], F32, tag="rk_sb")

            # ================= Stage 1: load x, LN1 =================
            for ti, (t0, tp) in enumerate(TT):
                nc.sync.dma_start(out=xres[:tp, ti], in_=x_d[t0:t0 + tp])
            for ti, (t0, tp) in enumerate(TT):
                xln_t = ev.tile([128, D], BF16, tag="xln_t")
                ln_tile(xres[:tp, ti], tp, xln_t[:tp], "ln1_g")
                pe_transpose_tile(tr_a, xln_t, tp, xT8, t0, f"trx{ti}")

            if stop_after <= 1:
                raise _StopBuild
            # ================= Stage 2: QKV matmul =================
            with tc.tile_pool(name="qkv_w", bufs=6) as wpool, \
                 tc.tile_pool(name="qkv_ps", bufs=4, space="PSUM") as pspool:
                wq_r = wqkv_d[:].rearrange("(ko p) n -> p ko n", p=128)
                for n0, nw in _nsplit(3 * D):
                    wt = wpool.tile([128, KD, 512], WDT, tag="wt")
                    nc.sync.dma_start(out=wt[:, :, :nw], in_=wq_r[:, :, n0:n0 + nw])
                    for ti, (t0, tp) in enumerate(TT):
                        ps = pspool.tile([128, 512], F32, tag="ps")
                        for k in range(0, KD, KSTEP):
                            nc.tensor.matmul(
                                ps[:tp, :nw],
                                xT8[:, k:k + KSTEP, t0:t0 + tp],
                                wt[:, k:k + KSTEP, :nw],
                                start=(k == 0),
                                stop=(k == KD - KSTEP),
                                perf_mode=DRM,
                            )
                        if fp8:
                            nc.any.tensor_scalar_mul(
                                qkv[:tp, ti, n0:n0 + nw], ps[:tp, :nw], descale
                            )
                        else:
                            nc.any.tensor_copy(
                                out=qkv[:tp, ti, n0:n0 + nw], in_=ps[:tp, :nw]
                            )

            if stop_after <= 2:
                raise _StopBuild
            # ================= Stage 3: QK-LN, build v_aug =================
            inv_hd = 1.0 / HD
            scale = HD ** (-0.5)
            for ti, (t0, tp) in enumerate(TT):
                # v | ones
                nc.vector.memset(v_aug[:tp, ti, :, HD:], 1.0)
                if tp < 128:
                    nc.vector.memset(v_aug[tp:, ti, :, HD:], 0.0)
                nc.gpsimd.tensor_copy(
                    out=v_aug[:tp, ti, :, :HD],
                    in_=qkv[:tp, ti, 2 * D:3 * D].rearrange("p (h d) -> p h d", h=H),
                )
                qk_fast = "qg" not in rep
                for which, base, gkey in (
                    ("q", 0, "qg"),
                    ("k", D, "kg"),
                ):
                    src = qkv[:tp, ti, base:base + D].rearrange(
                        "p (h d) -> p h d", h=H
                    )
                    sq = stats.tile([128, H, HD], F32, tag="qk_sq", bufs=1)
                    nc.gpsimd.tensor_mul(sq[:tp], src, src)
                    s1 = stats.tile([128, H], F32, tag="qk_s1")
                    nc.vector.reduce_sum(out=s1[:tp], in_=src, axis=AX)
                    s2 = stats.tile([128, H], F32, tag="qk_s2")
                    nc.vector.reduce_sum(out=s2[:tp], in_=sq[:tp], axis=AX)
                    mean = stats.tile([128, H], F32, tag="qk_mean")
                    nc.vector.tensor_scalar_mul(mean[:tp], s1[:tp], inv_hd)
                    msq = stats.tile([128, H], F32, tag="qk_msq")
                    nc.vector.tensor_mul(msq[:tp], mean[:tp], mean[:tp])
                    var = stats.tile([128, H], F32, tag="qk_var")
                    nc.vector.tensor_scalar(
                        out=var[:tp], in0=s2[:tp], scalar1=inv_hd, scalar2=None,
                        op0=OP.mult,
                    )
                    nc.vector.tensor_sub(var[:tp], var[:tp], msq[:tp])
                    sd = stats.tile([128, H], F32, tag="qk_sd")
                    nc.scalar.activation(
                        out=sd[:tp], in_=var[:tp], func=ACTF.Sqrt, bias=eps_t[:tp]
                    )
                    lnt = ev.tile([128, H, HD], BF16, tag="qk_out")
                    if qk_fast:
                        # k is centered only (rk folded into exp scale);
                        # q is scaled by rstd*hd^-0.5 only (its mean term
                        # vanishes against centered k).
                        if which == "k":
                            nc.vector.reciprocal(
                                out=rk_sb[:tp, ti], in_=sd[:tp]
                            )
                            nc.gpsimd.tensor_tensor(
                                lnt[:tp], src,
                                mean[:tp, :, None].to_broadcast([tp, H, HD]),
                                OP.subtract,
                            )
                        else:
                            rq = stats.tile([128, H], F32, tag="qk_rq")
                            nc.vector.reciprocal(out=rq[:tp], in_=sd[:tp])
                            nc.vector.tensor_scalar_mul(rq[:tp], rq[:tp], scale)
                            nc.vector.tensor_tensor(
                                lnt[:tp], src,
                                rq[:tp, :, None].to_broadcast([tp, H, HD]),
                                OP.mult,
                            )
                    else:
                        rstd = stats.tile([128, H], F32, tag="qk_rstd")
                        nc.vector.reciprocal(out=rstd[:tp], in_=sd[:tp])
                        nc.gpsimd.tensor_tensor(
                            lnt[:tp], src,
                            mean[:tp, :, None].to_broadcast([tp, H, HD]),
                            OP.subtract,
                        )
                        nc.vector.tensor_tensor(
                            lnt[:tp], lnt[:tp],
                            rstd[:tp, :, None].to_broadcast([tp, H, HD]), OP.mult,
                        )
                        g = rep[gkey]
                        b = rep["qb" if which == "q" else "kb"]
                        nc.vector.tensor_tensor(
                            lnt[:tp], lnt[:tp],
                            g[:tp, None, :].to_broadcast([tp, H, HD]), OP.mult,
                        )
                        nc.vector.tensor_tensor(
                            lnt[:tp], lnt[:tp],
                            b[:tp, None, :].to_broadcast([tp, H, HD]), OP.add,
                        )
                        if which == "q":
                            nc.vector.tensor_scalar_mul(lnt[:tp], lnt[:tp], scale)
                    flat = lnt[:tp].rearrange("p h d -> p (h d)")
                    dstT = qT if which == "q" else kT
                    pe_transpose_tile(tr_a, flat, tp, dstT, t0, f"tr{which}{ti}")

            # qkv dead; free its SBUF and start MLP weight streams into it
            qkv_cm.__exit__(None, None, None)
            mlp_w = ctx.enter_context(
                tc.tile_pool(name="mlp_w", bufs=1, side="right")
            )
            w1_r = w1_d[:].rearrange("(ko p) n -> p ko n", p=128)
            w1_tiles = []
            for ni, (n0, nw) in enumerate(_nsplit(HID)):
                wt = mlp_w.tile([128, KD, 512], WDT, tag=f"w1_{ni}",
                                name=f"w1t{ni}")
                nc.sync.dma_start(out=wt[:], in_=w1_r[:, :, n0:n0 + nw])
                w1_tiles.append(wt)
            wt2 = mlp_w.tile([128, KH, D], WDT, tag="wt2")
            nc.sync.dma_start(
                out=wt2[:], in_=w2_d[:].rearrange("(ko p) n -> p ko n", p=128)
            )

            tr_a_cm.__exit__(None, None, None)

            # ================= Stage 4: attention =================
            with tc.tile_pool(name="att_exp", bufs=2) as exp_pool, \
                 tc.tile_pool(name="att_ps", bufs=3, space="PSUM") as qk_ps_pool, \
                 tc.tile_pool(name="av_ps", bufs=2, space="PSUM") as av_ps_pool:
                def qk_exp(h):
                    c, off = h // 2, (h % 2) * 64
                    q_h = qT[off:off + 64, c]
                    k_h = kT[off:off + 64, c]
                    exp_tiles = []
                    for tk, (tk0, tkw) in enumerate(TT):
                        ps = qk_ps_pool.tile(
                            [128, T], F32, tag="qk_ps", name=f"qkps{h}_{tk}"
                        )
                        for n0, nw in _nsplit(T):
                            nc.tensor.matmul(
                                ps[:tkw, n0:n0 + nw],
                                k_h[:, tk0:tk0 + tkw],
                                q_h[:, n0:n0 + nw],
                                start=True,
                                stop=True,
                            )
                        et = exp_pool.tile(
                            [128, T], BF16, tag=f"exp{tk}", name=f"exp{h}_{tk}"
                        )
                        if "qg" not in rep:
                            nc.scalar.activation(
                                out=et[:tkw], in_=ps[:tkw], func=ACTF.Exp,
                                scale=rk_sb[:tkw, tk, h:h + 1],
                            )
                        else:
                            nc.scalar.activation(
                                out=et[:tkw], in_=ps[:tkw], func=ACTF.Exp
                            )
                        exp_tiles.append(et)
                    return exp_tiles

                def av(h, exp_tiles):
                    for mi, (m0, mp) in enumerate(TT):
                        pso = av_ps_pool.tile(
                            [128, HD + 1], F32, tag="av_ps", name=f"avps{h}_{mi}"
                        )
                        for tk, (tk0, tkw) in enumerate(TT):
                            nc.tensor.matmul(
                                pso[:mp],
                                exp_tiles[tk][:tkw, m0:m0 + mp],
                                v_aug[:tkw, tk, h],
                                start=(tk == 0),
                                stop=(tk == NT - 1),
                            )
                        rc = stats.tile([128, 1], F32, tag="att_rc")
                        nc.vector.reciprocal(out=rc[:mp], in_=pso[:mp, HD:])
                        nc.vector.tensor_scalar_mul(
                            attn[:mp, mi, h * HD:(h + 1) * HD],
                            pso[:mp, :HD], rc[:mp],
                        )

                prev = None
                for h in range(H):
                    cur = qk_exp(h)
                    if prev is not None:
                        av(h - 1, prev)
                    prev = cur
                av(H - 1, prev)
            qkT_cm.__exit__(None, None, None)

            tr_b = ctx.enter_context(
                tc.tile_pool(name="tr_b", bufs=2, space="PSUM", side="right")
            )
            # ================= Stage 5: o-LN =================
            for ti, (t0, tp) in enumerate(TT):
                ot = ev.tile([128, D], BF16, tag="oln_t")
                ln_tile(attn[:tp, ti], tp, ot[:tp], "o_g")
                pe_transpose_tile(tr_b, ot, tp, oT8, t0, f"tro{ti}")

            if stop_after <= 5:
                raise _StopBuild
            # ================= Stage 6: proj + residual =================
            with tc.tile_pool(name="proj_w", bufs=2) as wpool, \
                 tc.tile_pool(name="proj_ps", bufs=4, space="PSUM") as pspool:
                wp_r = wproj_d[:].rearrange("(ko p) n -> p ko n", p=128)
                for n0, nw in _nsplit(D):
                    wt = wpool.tile([128, KD, 512], WDT, tag="wt")
                    nc.sync.dma_start(out=wt[:, :, :nw], in_=wp_r[:, :, n0:n0 + nw])
                    for ti, (t0, tp) in enumerate(TT):
                        ps = pspool.tile([128, 512], F32, tag="ps")
                        for k in range(0, KD, KSTEP):
                            nc.tensor.matmul(
                                ps[:tp, :nw],
                                oT8[:, k:k + KSTEP, t0:t0 + tp],
                                wt[:, k:k + KSTEP, :nw],
                                start=(k == 0),
                                stop=(k == KD - KSTEP),
                                perf_mode=DRM,
                            )
                        dst = r1[:tp, ti, n0:n0 + nw]
                        if fp8:
                            if "ls1s" in opt_d:
                                nc.scalar.activation(
                                    out=dst, in_=ps[:tp, :nw],
                                    func=ACTF.Identity,
                                    scale=ls_sb["ls1s"][:tp],
                                )
                            else:
                                nc.vector.tensor_mul(
                                    dst, ps[:tp, :nw],
                                    ls_sb["ls1v"][:tp, n0:n0 + nw],
                                )
                            nc.vector.tensor_add(
                                dst, dst, xres[:tp, ti, n0:n0 + nw]
                            )
                        else:
                            nc.vector.tensor_add(
                                dst, ps[:tp, :nw], xres[:tp, ti, n0:n0 + nw]
                            )
                        if "bproj" in rep:
                            nc.vector.tensor_add(
                                dst, dst, rep["bproj"][:tp, n0:n0 + nw]
                            )

        if stop_after <= 6:
            raise _StopBuild
        # ================= Stage 7: LN2 =================
        for ti, (t0, tp) in enumerate(TT):
            lt = ev.tile([128, D], BF16, tag="ln2_t")
            ln_tile(r1[:tp, ti], tp, lt[:tp], "ln2_g")
            pe_transpose_tile(tr_b, lt, tp, l2T8, t0, f"trl{ti}")

        if stop_after <= 7:
            raise _StopBuild
        # ============ Stage 8+9: MLP (feature-major hidden) ============
        with tc.tile_pool(name="mlp_sb", bufs=1) as mlp_sb, \
             tc.tile_pool(name="m1_ps", bufs=2, space="PSUM") as ps1pool, \
             tc.tile_pool(name="m2_ps", bufs=2, space="PSUM") as ps2pool, \
             tc.tile_pool(name="m2_out", bufs=2) as opool:
            hT = mlp_sb.tile([128, KH, T], WDT, tag="hT")
            b1_fm = None
            if flags.get("b1"):
                b1_fm = mlp_sb.tile([128, KH], F32, tag="b1_fm")
                nc.sync.dma_start(
                    out=b1_fm, in_=opt_d["b1"][:].rearrange("(c p) -> p c", p=128)
                )
            # MLP1: out chunk mh (128 HID dims) = gelu(w1_chunk^T @ ln2^T)
            for ni, (n0, nw) in enumerate(_nsplit(HID)):
                wt = w1_tiles[ni]
                for j in range(4):
                    mh = ni * 4 + j
                    ps = ps1pool.tile([128, T], F32, tag="ps1")
                    for k in range(0, KD, KSTEP):
                        for s0, sw in _nsplit(T):
                            nc.tensor.matmul(
                                ps[:, s0:s0 + sw],
                                wt[:, k:k + KSTEP, j * 128:(j + 1) * 128],
                                l2T8[:, k:k + KSTEP, s0:s0 + sw],
                                start=(k == 0),
                                stop=(k == KD - KSTEP),
                                perf_mode=DRM,
                            )
                    bias = b1_fm[:, mh:mh + 1] if b1_fm is not None else 0.0
                    nc.scalar.activation(out=hT[:, mh], in_=ps[:], func=gelu,
                                         bias=bias, scale=descale)
            # MLP2: token-major out (w2 preloaded during attention)
            for ti, (t0, tp) in enumerate(TT):
                for n0, nw in _nsplit(D):
                    ps = ps2pool.tile([128, 512], F32, tag="ps2")
                    for k in range(0, KH, KSTEP):
                        nc.tensor.matmul(
                            ps[:tp],
                            hT[:, k:k + KSTEP, t0:t0 + tp],
                            wt2[:, k:k + KSTEP, n0:n0 + nw],
                            start=(k == 0),
                            stop=(k == KH - KSTEP),
                            perf_mode=DRM,
                        )
                    ot = opool.tile([128, 512], F32, tag="ot")
                    if fp8:
                        if "ls2s" in opt_d:
                            nc.scalar.activation(
                                out=ot[:tp], in_=ps[:tp], func=ACTF.Identity,
                                scale=ls_sb["ls2s"][:tp],
                            )
                        else:
                            nc.vector.tensor_mul(
                                ot[:tp], ps[:tp], ls_sb["ls2v"][:tp, n0:n0 + nw]
                            )
                        nc.vector.tensor_add(
                            ot[:tp], ot[:tp], r1[:tp, ti, n0:n0 + nw]
                        )
                    else:
                        nc.vector.tensor_add(
                            ot[:tp], ps[:tp], r1[:tp, ti, n0:n0 + nw]
                        )
                    if "b2" in rep:
                        nc.vector.tensor_add(
                            ot[:tp], ot[:tp], rep["b2"][:tp, n0:n0 + nw]
                        )
                    nc.scalar.dma_start(
                        out=out_d[t0:t0 + tp, n0:n0 + nw], in_=ot[:tp]
                    )


def _nontrivial(a, val):
    return not np.allclose(a, val, rtol=0.0, atol=0.0)


def prepare(inputs, fp8=True):
    """Host-side preprocessing: dtype casts and scale folding.

    Returns (flags, common_map) where common_map holds every device input
    except per-core "x".

    bf16 path: ls1/ls2 are folded into w_proj/w2.
    fp8 path: weights are scaled by W_SCALE (so sigma~0.32 stays in e4m3
    normal range; ls*1e-5 would underflow), and ls/W_SCALE is applied at
    eviction via the ls1s/ls2s (uniform) or ls1v/ls2v (vector) inputs.
    """
    f32 = np.float32
    bf16 = ml_dtypes.bfloat16
    w_qkv = np.asarray(inputs["w_qkv"], f32)
    w_proj = np.asarray(inputs["w_proj"], f32)
    w1 = np.asarray(inputs["w1"], f32)
    w2 = np.asarray(inputs["w2"], f32)
    ls1 = np.asarray(inputs["ls1"], f32)
    ls2 = np.asarray(inputs["ls2"], f32)

    flags = {
        "ln1": _nontrivial(inputs["ln1_g"], 1) or _nontrivial(inputs["ln1_b"], 0),
        "ln2": _nontrivial(inputs["ln2_g"], 1) or _nontrivial(inputs["ln2_b"], 0),
        "oln": _nontrivial(inputs["o_g"], 1) or _nontrivial(inputs["o_b"], 0),
        "qk": _nontrivial(inputs["q_g"], 1) or _nontrivial(inputs["q_b"], 0)
        or _nontrivial(inputs["k_g"], 1) or _nontrivial(inputs["k_b"], 0),
        "bproj": _nontrivial(inputs["b_proj"] * ls1, 0),
        "b1": _nontrivial(inputs["b1"], 0),
        "b2": _nontrivial(inputs["b2"] * ls2, 0),
    }

    if fp8:
        e4 = mybir.dt.np(FP8)
        flags["ls1u"] = bool(np.all(ls1 == ls1[0]))
        flags["ls2u"] = bool(np.all(ls2 == ls2[0]))
        cm = {
            "wqkv": (w_qkv * W_SCALE).astype(e4),
            "wproj": (w_proj * W_SCALE).astype(e4),
            "w1": (w1 * W_SCALE).astype(e4),
            "w2": (w2 * W_SCALE).astype(e4),
        }
        if flags["ls1u"]:
            cm["ls1s"] = (ls1[:1] / W_SCALE).astype(f32)
        else:
            cm["ls1v"] = (ls1 / W_SCALE).astype(f32)
        if flags["ls2u"]:
            cm["ls2s"] = (ls2[:1] / W_SCALE).astype(f32)
        else:
            cm["ls2v"] = (ls2 / W_SCALE).astype(f32)
    else:
        cm = {
            "wqkv": w_qkv.astype(bf16),
            "wproj": (w_proj * ls1[None, :]).astype(bf16),
            "w1": w1.astype(bf16),
            "w2": (w2 * ls2[None, :]).astype(bf16),
        }
    if flags["ln1"]:
        cm["ln1_g"] = np.asarray(inputs["ln1_g"], f32)
        cm["ln1_b"] = np.asarray(inputs["ln1_b"], f32)
    if flags["ln2"]:
        cm["ln2_g"] = np.asarray(inputs["ln2_g"], f32)
        cm["ln2_b"] = np.asarray(inputs["ln2_b"], f32)
    if flags["oln"]:
        cm["o_g"] = np.asarray(inputs["o_g"], f32)
        cm["o_b"] = np.asarray(inputs["o_b"], f32)
    if flags["qk"]:
        cm["qg"] = np.asarray(inputs["q_g"], f32)
        cm["qb"] = np.asarray(inputs["q_b"], f32)
        cm["kg"] = np.asarray(inputs["k_g"], f32)
        cm["kb"] = np.asarray(inputs["k_b"], f32)
    if flags["bproj"]:
        cm["bproj"] = (np.asarray(inputs["b_proj"], f32) * ls1).astype(f32)
    if flags["b1"]:
        cm["b1"] = np.asarray(inputs["b1"], f32)
    if flags["b2"]:
        cm["b2"] = (np.asarray(inputs["b2"], f32) * ls2).astype(f32)
    return flags, cm


_CACHE = {}


def get_compiled(flags, fp8=True):
    key = (fp8,) + tuple(sorted((k, bool(v)) for k, v in flags.items()))
    if key not in _CACHE:
        _CACHE[key] = build_block(T=576, flags=flags, fp8=fp8)
    return _CACHE[key]


def kernel(**inputs):
    from concourse import bass_utils

    x = np.asarray(inputs["x"], np.float32)
    B = x.shape[0]
    assert B == N_CORES
    fp8 = True
    flags, cm = prepare(inputs, fp8=fp8)
    nc = get_compiled(flags, fp8=fp8)
    in_maps = [dict(cm, x=np.ascontiguousarray(x[i])) for i in range(B)]
    res = bass_utils.run_bass_kernel_spmd(nc, in_maps, core_ids=list(range(B)))
    out = np.stack([res.results[i]["out"] for i in range(B)], axis=0)
    return out.astype(np.float32)


if __name__ == "__main__":
    import reference

    inputs = {k: np.asarray(v) for k, v in reference.setup_inputs().items()}
    expected = np.asarray(reference.reference(**reference.setup_inputs()))
    actual = kernel(**inputs)
    err = np.linalg.norm(actual - expected) / np.linalg.norm(expected)
    print("Relative error:", err)
